# revision 1
# baseline (speedup 1.0000x reference)
"""AxialTransformerBlock Trainium2 kernel (8 NeuronCores, SPMD + AllToAll).

Sharding: sequence-parallel over S (512 rows/core) for LN / channel attention /
MLP; head-parallel via AllToAll for temporal causal attention (8 (c,h) pairs
per core over the full sequence), AllToAll back for the output projection.

On-device layout: feature-major residual stream x^T [D, T] so every GEMM uses
fp32r matmuls with no activation transposes. Host pre-transposes x and all
weights, bakes RoPE cos/sin tables (with even/odd de-interleave folded into the
Wq_t/Wk_t column permutation) and the causal / channel block-diagonal masks.
"""

import numpy as np

N_CORES = 8
S, C, D = 4096, 4, 1024
SB = S // N_CORES          # 512 s-rows per core
TL = SB * C                # 2048 local tokens
H_T, HD_T = 16, 64
H_C, HD_C = 4, 256
F_MLP = 4 * D              # 4096
LN_EPS = 1e-5
TC = 512                   # token chunk for phases A/B
NCH = TL // TC             # 4
MC2 = 256                  # MLP second-gemm chunk

_CACHE = {}


def _build_program():
    import concourse.bass as bass
    import concourse.bacc as bacc
    import concourse.tile as tile
    from concourse import mybir

    F32 = mybir.dt.float32
    F32R = mybir.dt.float32r
    AF = mybir.ActivationFunctionType
    OP = mybir.AluOpType
    ts = bass.ts

    nc = bacc.Bacc("TRN2", target_bir_lowering=False, debug=False,
                   num_devices=N_CORES)

    def din(name, shape):
        return nc.dram_tensor(name, list(shape), F32, kind="ExternalInput").ap()

    xT = din("xT", [D, TL])
    wqcT = din("wqcT", [D, D])
    wkcT = din("wkcT", [D, D])
    wvcT = din("wvcT", [D, D])
    wocT = din("wocT", [D, D])
    wqtT = din("wqtT", [D, D])
    wktT = din("wktT", [D, D])
    wvtT = din("wvtT", [D, D])
    wotT = din("wotT", [D, D])
    w1T = din("w1T", [D, F_MLP])
    w2T = din("w2T", [F_MLP, D])
    gb_c = din("gb_c", [D, 2])   # col0 = g, col1 = b
    gb_t = din("gb_t", [D, 2])
    gb_m = din("gb_m", [D, 2])
    b1v = din("b1v", [F_MLP, 1])
    b2v = din("b2v", [D, 1])
    cq_d = din("cq", [128, TC])
    sq_d = din("sq", [128, TC])
    ck_d = din("ck", [128, TC])
    sk_d = din("sk", [128, TC])
    mkc_d = din("mkc", [128, 128])
    mkt_d = din("mkt", [4, 128, TC])
    id_d = din("idm", [128, 128])

    yT = nc.dram_tensor("yT", [D, TL], F32, kind="ExternalOutput").ap()

    # internal DRAM
    import os
    dbg = os.environ.get("KDBG", "0") == "1"
    kindd = "ExternalOutput" if dbg else "Internal"
    PH = set(os.environ.get("KPHASES", "A,Bf,Ba,Bo,M1,M2").split(","))
    x1cm = nc.dram_tensor("x1cm", [D, TL], F32R, kind=kindd).ap()
    x2cm = nc.dram_tensor("x2cm", [D, TL], F32R, kind=kindd).ap()
    hbuf = nc.dram_tensor("hbuf", [F_MLP, TL], F32R, kind=kindd).ap()
    a2aQi = nc.dram_tensor("a2aQi", [8, 8, 64, TC], F32).ap()
    a2aQo = nc.dram_tensor("a2aQo", [8, 8, 64, TC], F32).ap()
    a2aKi = nc.dram_tensor("a2aKi", [8, 8, 64, TC], F32).ap()
    a2aKo = nc.dram_tensor("a2aKo", [8, 8, 64, TC], F32).ap()
    a2aVi = nc.dram_tensor("a2aVi", [8, 8, TC, 64], F32).ap()
    a2aVo = nc.dram_tensor("a2aVo", [8, 8, TC, 64], F32).ap()
    a2aAi = nc.dram_tensor("a2aAi", [8, 8, 64, TC], F32).ap()
    a2aAo = nc.dram_tensor("a2aAo", [8, 8, 64, TC], F32).ap()

    RG = [list(range(N_CORES))]

    def kpe(w):  # [D_in, E] dram -> [p, kt, e] view
        return w.bitcast(F32R).rearrange("(k p) e -> p k e", p=128)

    with tile.TileContext(nc) as tc:
        cst_cm = tc.tile_pool(name="cst", bufs=1)
        cst = cst_cm.__enter__()
        ones1f = cst.tile([128, 1], F32)
        nc.vector.memset(ones1f, 1.0)
        ones1 = ones1f.bitcast(F32R)
        eps1 = cst.tile([1, 1], F32)
        nc.vector.memset(eps1, LN_EPS)
        gbc_sb = cst.tile([128, 8, 2], F32)
        nc.sync.dma_start(out=gbc_sb, in_=gb_c.rearrange("(k p) two -> p k two", p=128))
        gbt_sb = cst.tile([128, 8, 2], F32)
        nc.sync.dma_start(out=gbt_sb, in_=gb_t.rearrange("(k p) two -> p k two", p=128))
        gbm_sb = cst.tile([128, 8, 2], F32)
        nc.sync.dma_start(out=gbm_sb, in_=gb_m.rearrange("(k p) two -> p k two", p=128))
        b1_sb = cst.tile([128, 32], F32)
        nc.sync.dma_start(out=b1_sb, in_=b1v.rearrange("(k p) one -> p (k one)", p=128))
        b2_sb = cst.tile([128, 8], F32)
        nc.sync.dma_start(out=b2_sb, in_=b2v.rearrange("(k p) one -> p (k one)", p=128))

        def layernorm(pool, psum, x_ch, gb_sb, width):
            """x_ch [128, 8, width] f32r -> n_ch same shape; returns n_ch."""
            nkt = 8
            stat_ps = psum.tile([1, width], F32, tag="stat_ps",
                                name="stat_ps", bufs=1)
            stat_ps2 = psum.tile([1, width], F32, tag="stat_ps2",
                                 name="stat_ps2", bufs=1)
            for kt in range(nkt):
                nc.tensor.matmul(stat_ps, ones1, x_ch[:, kt, :],
                                 start=(kt == 0), stop=(kt == nkt - 1))
            for kt in range(nkt):
                xsq = pool.tile([128, width], F32R, tag="ln_xsq", name="xsq")
                nc.scalar.activation(xsq, x_ch[:, kt, :], AF.Square)
                nc.tensor.matmul(stat_ps2, ones1, xsq,
                                 start=(kt == 0), stop=(kt == nkt - 1))
            mu = pool.tile([1, width], F32, tag="ln_mu", name="mu")
            nc.vector.tensor_scalar_mul(mu, stat_ps, 1.0 / D)
            ex2 = pool.tile([1, width], F32, tag="ln_ex2", name="ex2")
            nc.vector.tensor_scalar_mul(ex2, stat_ps2, 1.0 / D)
            var = pool.tile([1, width], F32, tag="ln_var", name="var")
            nc.vector.tensor_tensor(var, mu, mu, OP.mult)
            nc.vector.tensor_tensor(var, ex2, var, OP.subtract)
            sd = pool.tile([1, width], F32, tag="ln_sd", name="sd")
            nc.scalar.activation(sd, var, AF.Sqrt, bias=eps1)
            rs = pool.tile([1, width], F32, tag="ln_rs", name="rs")
            nc.vector.reciprocal(rs, sd)
            bv = pool.tile([1, width], F32, tag="ln_bv", name="bv")
            nc.vector.tensor_tensor(bv, mu, rs, OP.mult)
            ab = pool.tile([128, width], F32R, tag="ln_ab", name="ab")
            nc.gpsimd.partition_broadcast(ab, rs.bitcast(F32R))
            bb = pool.tile([128, width], F32R, tag="ln_bb", name="bb")
            nc.gpsimd.partition_broadcast(bb, bv.bitcast(F32R))
            n_ch = pool.tile([128, 8, width], F32R, tag="ln_out",
                             name="n_ch", bufs=1)
            for kt in range(nkt):
                t1 = pool.tile([128, width], F32R, tag="ln_t1", name="t1")
                nc.vector.tensor_tensor(t1, x_ch[:, kt, :], ab, OP.mult)
                nc.vector.tensor_tensor(t1, t1, bb, OP.subtract)
                nc.vector.tensor_scalar(n_ch[:, kt, :], t1,
                                        gb_sb[:, kt, 0:1], gb_sb[:, kt, 1:2],
                                        OP.mult, OP.add)
            return n_ch

        def proj_fmajor(pool, psum, wdram, n_ch, out_ch, width):
            """out_ch[:, et, :] = W_et^T @ n  (feature-major out)."""
            for et in range(8):
                w_t = pool.tile([128, 8, 128], F32R, tag="wstream", name="w_t",
                                bufs=3)
                nc.sync.dma_start(out=w_t, in_=kpe(wdram)[:, :, ts(et, 128)])
                ps = psum.tile([128, width], F32, tag="ps_proj", name="ps",
                               bufs=2)
                for kt in range(8):
                    nc.tensor.matmul(ps, w_t[:, kt, :], n_ch[:, kt, :],
                                     start=(kt == 0), stop=(kt == 7))
                nc.scalar.activation(out_ch[:, et, :], ps, AF.Copy)

        def proj_tmajor(pool, psum, wdram, n_ch, out_ch, width):
            """V token-major: out_ch [128, width//128, 1024]; k-outer with
            width//128 live psum banks so Wv streams in small tiles."""
            ntt = width // 128
            for ec in range(2):
                psv = [psum.tile([128, 512], F32, tag=f"psv{i}",
                                 name=f"psv{i}", bufs=1) for i in range(ntt)]
                for kt in range(8):
                    w_t = pool.tile([128, 512], F32R, tag="wstreamV",
                                    name="w_tv", bufs=3)
                    nc.sync.dma_start(out=w_t,
                                      in_=kpe(wdram)[:, kt, ts(ec, 512)])
                    for tt_ in range(ntt):
                        nc.tensor.matmul(psv[tt_], n_ch[:, kt, ts(tt_, 128)],
                                         w_t, start=(kt == 0), stop=(kt == 7))
                for tt_ in range(ntt):
                    nc.scalar.activation(out_ch[:, tt_, ts(ec, 512)], psv[tt_],
                                         AF.Copy)

        # ---------------- Phase A: channel attention ----------------
        if "A" in PH:
         with (tc.tile_pool(name="pa", bufs=2) as pa,
              tc.tile_pool(name="pa1", bufs=1) as pa1,
              tc.tile_pool(name="pa_ps", bufs=1, space="PSUM") as pa_ps):
             mkc_sb = pa1.tile([128, 128], F32, tag="mkc", name="mkc_sb")
             nc.sync.dma_start(out=mkc_sb, in_=mkc_d)
             id_sb = pa1.tile([128, 128], F32, tag="idm", name="id_sb")
             nc.sync.dma_start(out=id_sb, in_=id_d)
             for ch in range(NCH):
                 x_ch = pa1.tile([128, 8, TC], F32R, tag="x_ch", name="x_ch")
                 nc.sync.dma_start(
                     out=x_ch,
                     in_=xT.bitcast(F32R).rearrange("(k p) t -> p k t", p=128)[:, :, ts(ch, TC)])
                 n_ch = layernorm(pa, pa_ps, x_ch, gbc_sb, TC)
                 q_ch = pa1.tile([128, 8, TC], F32R, tag="q_ch", name="q_ch")
                 proj_fmajor(pa, pa_ps, wqcT, n_ch, q_ch, TC)
                 k_ch = pa1.tile([128, 8, TC], F32R, tag="k_ch", name="k_ch")
                 proj_fmajor(pa, pa_ps, wkcT, n_ch, k_ch, TC)
                 v_ch = pa1.tile([128, 4, 1024], F32R, tag="v_ch", name="v_ch")
                 proj_tmajor(pa, pa_ps, wvcT, n_ch, v_ch, TC)

                 # attention (block-diag over 4 channels, s-major tokens)
                 pTs = {}
                 for h in range(H_C):
                     for qt in range(4):
                         ps_s = pa_ps.tile([128, 128], F32, tag="psv0",
                                           name="ps_s", bufs=1)
                         for hf in range(2):
                             et = h * 2 + hf
                             nc.tensor.matmul(ps_s, q_ch[:, et, ts(qt, 128)],
                                              k_ch[:, et, ts(qt, 128)],
                                              start=(hf == 0), stop=(hf == 1))
                         pe = pa.tile([128, 128], F32, tag="pe", name="pe")
                         nc.scalar.activation(pe, ps_s, AF.Exp, scale=1.0 / 16.0)
                         pm = pa.tile([128, 128], F32, tag="pm", name="pm")
                         den = pa.tile([128, 1], F32, tag="den", name="den")
                         nc.vector.tensor_tensor(pm, pe, mkc_sb, OP.mult)
                         nc.vector.reduce_sum(den, pm, axis=mybir.AxisListType.X)
                         rec = pa.tile([128, 1], F32, tag="rec", name="rec")
                         nc.vector.reciprocal(rec, den)
                         nc.vector.tensor_scalar_mul(pm, pm, rec)
                         ps_t = pa_ps.tile([128, 128], F32, tag="psv1",
                                           name="ps_t", bufs=1)
                         nc.tensor.transpose(ps_t, pm, id_sb)
                         pT = pa1.tile([128, 128], F32R, tag=f"pT_{h}_{qt}",
                                       name=f"pT_{h}_{qt}")
                         nc.scalar.activation(pT, ps_t, AF.Copy)
                         pTs[(h, qt)] = pT
                 aT_ch = pa.tile([128, 8, TC], F32R, tag="ln_out",
                                 name="aT_ch", bufs=1)
                 for es in range(8):
                     ps_av = pa_ps.tile([128, TC], F32, tag="psv2",
                                        name="ps_av", bufs=1)
                     for qt in range(4):
                         nc.tensor.matmul(ps_av[:, ts(qt, 128)],
                                          v_ch[:, qt, ts(es, 128)],
                                          pTs[(es // 2, qt)],
                                          start=True, stop=True)
                     nc.scalar.activation(aT_ch[:, es, :], ps_av, AF.Copy)
                 # Wo + residual, write c-major
                 for dt in range(8):
                     w_t = pa.tile([128, 8, 128], F32R, tag="wstream", name="w_t",
                                  bufs=3)
                     nc.sync.dma_start(out=w_t, in_=kpe(wocT)[:, :, ts(dt, 128)])
                     ps_o = pa_ps.tile([128, TC], F32, tag="psv3", name="ps_o",
                                       bufs=1)
                     for et in range(8):
                         nc.tensor.matmul(ps_o, w_t[:, et, :], aT_ch[:, et, :],
                                          start=(et == 0), stop=(et == 7))
                     x1t = pa.tile([128, 4, 128], F32R, tag="x1t", name="x1t")
                     nc.vector.tensor_tensor(
                         x1t.rearrange("p c s -> p s c"),
                         ps_o.rearrange("p (s c) -> p s c", c=4),
                         x_ch[:, dt, :].rearrange("p (s c) -> p s c", c=4),
                         OP.add)
                     nc.sync.dma_start(
                         out=x1cm[ts(dt, 128), :].rearrange("p (c s) -> p c s", c=4)[:, :, ts(ch, 128)],
                         in_=x1t)

        # ---------------- Phase B: temporal attention ----------------
        if PH & {"Bf", "Ba", "Bo"}:
         with (tc.tile_pool(name="pb", bufs=2) as pb,
              tc.tile_pool(name="pb1", bufs=1) as pb1,
              tc.tile_pool(name="pb_ps", bufs=1, space="PSUM") as pb_ps):
             cq_sb = pb1.tile([128, TC], F32R, tag="cq", name="cq_sb")
             nc.sync.dma_start(out=cq_sb, in_=cq_d.bitcast(F32R))
             sq_sb = pb1.tile([128, TC], F32R, tag="sq", name="sq_sb")
             nc.sync.dma_start(out=sq_sb, in_=sq_d.bitcast(F32R))
             ck_sb = pb1.tile([128, TC], F32R, tag="ck", name="ck_sb")
             nc.sync.dma_start(out=ck_sb, in_=ck_d.bitcast(F32R))
             sk_sb = pb1.tile([128, TC], F32R, tag="sk", name="sk_sb")
             nc.sync.dma_start(out=sk_sb, in_=sk_d.bitcast(F32R))
             mkt_sb = pb1.tile([128, 4, TC], F32R, tag="mkt", name="mkt_sb")
             nc.sync.dma_start(out=mkt_sb,
                               in_=mkt_d.bitcast(F32R).rearrange("k p q -> p k q"))
             for c in range(C if "Bf" in PH else 0):
                 x1_ch = pb1.tile([128, 8, TC], F32R, tag="x_ch", name="x1_ch")
                 nc.sync.dma_start(
                     out=x1_ch,
                     in_=x1cm.rearrange("(k p) t -> p k t", p=128)[:, :, ts(c, TC)])
                 n_ch = layernorm(pb, pb_ps, x1_ch, gbt_sb, TC)
                 q_ch = pb1.tile([128, 8, TC], F32R, tag="q_ch", name="q_ch")
                 proj_fmajor(pb, pb_ps, wqtT, n_ch, q_ch, TC)
                 k_ch = pb1.tile([128, 8, TC], F32R, tag="k_ch", name="k_ch")
                 proj_fmajor(pb, pb_ps, wktT, n_ch, k_ch, TC)
                 v_ch = pb1.tile([128, 4, 1024], F32R, tag="v_ch", name="v_ch")
                 proj_tmajor(pb, pb_ps, wvtT, n_ch, v_ch, TC)
                 # RoPE in place on q_ch/k_ch (ev tiles kt, od tiles kt+4)
                 for tgt, cos_sb, sin_sb in ((q_ch, cq_sb, sq_sb),
                                             (k_ch, ck_sb, sk_sb)):
                     for pr in range(4):
                         ev = tgt[:, pr, :]
                         od = tgt[:, pr + 4, :]
                         t1 = pb.tile([128, TC], F32R, tag="rp1", name="t1")
                         t2 = pb.tile([128, TC], F32R, tag="rp2", name="t2")
                         t3 = pb.tile([128, TC], F32R, tag="rp3", name="t3")
                         t4 = pb.tile([128, TC], F32R, tag="rp4", name="t4")
                         nc.vector.tensor_tensor(t1, ev, cos_sb, OP.mult)
                         nc.vector.tensor_tensor(t2, ev, sin_sb, OP.mult)
                         nc.vector.tensor_tensor(t3, od, sin_sb, OP.mult)
                         nc.vector.tensor_tensor(t4, od, cos_sb, OP.mult)
                         nc.vector.tensor_tensor(ev, t1, t3, OP.subtract)
                         nc.vector.tensor_tensor(od, t2, t4, OP.add)
                 # scatter to A2A send buffers
                 for h in range(H_T):
                     g = c * H_T + h
                     j, pl = g // 8, g % 8
                     prow = (h % 4) * 32
                     nc.sync.dma_start(out=a2aQi[j, pl, 0:32, :].bitcast(F32R),
                                       in_=q_ch[prow:prow + 32, h // 4, :])
                     nc.sync.dma_start(out=a2aQi[j, pl, 32:64, :].bitcast(F32R),
                                       in_=q_ch[prow:prow + 32, 4 + h // 4, :])
                     nc.sync.dma_start(out=a2aKi[j, pl, 0:32, :].bitcast(F32R),
                                       in_=k_ch[prow:prow + 32, h // 4, :])
                     nc.sync.dma_start(out=a2aKi[j, pl, 32:64, :].bitcast(F32R),
                                       in_=k_ch[prow:prow + 32, 4 + h // 4, :])
                     nc.sync.dma_start(
                         out=a2aVi[j, pl].bitcast(F32R).rearrange("(tt p) hd -> p tt hd", p=128),
                         in_=v_ch[:, :, ts(h, 64)])
             for src, dst in (((a2aQi, a2aQo), (a2aKi, a2aKo), (a2aVi, a2aVo)) if "Bf" in PH else ()):
                 nc.gpsimd.collective_compute(
                     "AllToAll", OP.bypass, replica_groups=RG,
                     ins=[src.opt()], outs=[dst.opt()])

             # flash attention per local pair over full S
             for p in range(8 if "Ba" in PH else 0):
                 kTp = pb1.tile([64, S], F32R, tag="x_ch", name="kTp")
                 for src in range(8):
                     nc.sync.dma_start(out=kTp[:, ts(src, TC)],
                                       in_=a2aKo[src, p].bitcast(F32R))
                 vp = pb1.tile([128, 32, 65], F32R, tag="q_ch", name="vp")
                 for kt in range(32):
                     nc.sync.dma_start(
                         out=vp[:, kt, 0:64],
                         in_=a2aVo[kt // 4, p].bitcast(F32R)[ts(kt % 4, 128), :])
                 nc.vector.memset(vp[:, :, 64:65].bitcast(F32), 1.0)
                 for qc in range(8):
                     qTp = pb.tile([64, TC], F32R, tag="qTp", name="qTp")
                     nc.sync.dma_start(out=qTp, in_=a2aQo[qc, p].bitcast(F32R))
                     ps_a = pb_ps.tile([128, TC], F32, tag=f"psv{2 + qc % 2}",
                                       name="ps_a", bufs=1)
                     nk = (qc + 1) * 4
                     for kt in range(nk):
                         ps_sc = pb_ps.tile([128, TC], F32, tag=f"psv{kt % 2}",
                                            name="ps_sc", bufs=1)
                         nc.tensor.matmul(ps_sc, kTp[:, ts(kt, 128)], qTp,
                                          start=True, stop=True)
                         pexp = pb.tile([128, TC], F32R, tag="pexp", name="pexp",
                                        bufs=3)
                         nc.scalar.activation(pexp, ps_sc, AF.Exp)
                         if kt >= qc * 4:
                             nc.vector.tensor_tensor(pexp, pexp,
                                                     mkt_sb[:, kt - qc * 4, :],
                                                     OP.mult)
                         nc.tensor.matmul(ps_a[0:65, :], vp[:, kt, :], pexp,
                                          start=(kt == 0), stop=(kt == nk - 1))
                     rec1 = pb.tile([1, TC], F32, tag="rec1", name="rec1")
                     nc.vector.reciprocal(rec1, ps_a[64:65, :])
                     rb = pb.tile([64, TC], F32R, tag="rb", name="rb")
                     nc.gpsimd.partition_broadcast(rb, rec1.bitcast(F32R))
                     aT = pb.tile([64, TC], F32, tag="aT", name="aT")
                     nc.vector.tensor_tensor(aT, ps_a[0:64, :], rb, OP.mult)
                     nc.sync.dma_start(out=a2aAi[qc, p], in_=aT)
             if "Ba" in PH:
                 nc.gpsimd.collective_compute(
                     "AllToAll", OP.bypass, replica_groups=RG,
                     ins=[a2aAi.opt()], outs=[a2aAo.opt()])

             # Wo_t + residual per channel
             for c in range(C if "Bo" in PH else 0):
                 for dt in range(8):
                     w_t = pb.tile([128, 8, 128], F32R, tag="wstream", name="w_t",
                                   bufs=3)
                     nc.sync.dma_start(out=w_t, in_=kpe(wotT)[:, :, ts(dt, 128)])
                     ps_o = pb_ps.tile([128, TC], F32, tag="ps_proj",
                                       name="ps_o", bufs=2)
                     for et in range(8):
                         g0 = c * H_T + et * 2
                         rhsA = pb.tile([128, TC], F32R, tag="rhsA", name="rhsA")
                         nc.sync.dma_start(
                             out=rhsA,
                             in_=a2aAo[g0 // 8, g0 % 8:g0 % 8 + 2].bitcast(F32R).rearrange("a r q -> (a r) q"))
                         nc.tensor.matmul(ps_o, w_t[:, et, :], rhsA,
                                          start=(et == 0), stop=(et == 7))
                     x1c = pb.tile([128, TC], F32R, tag="x1c", name="x1c")
                     nc.sync.dma_start(out=x1c, in_=x1cm[ts(dt, 128), ts(c, TC)])
                     x2t = pb.tile([128, TC], F32R, tag="x2t", name="x2t")
                     nc.vector.tensor_tensor(x2t, ps_o, x1c, OP.add)
                     nc.sync.dma_start(out=x2cm[ts(dt, 128), ts(c, TC)], in_=x2t)

        # ---------------- Phase C: MLP ----------------
        if "M1" in PH:
         with (tc.tile_pool(name="pm1", bufs=2) as pm1,
              tc.tile_pool(name="pm1b", bufs=1) as pm1b,
              tc.tile_pool(name="pm1_ps", bufs=2, space="PSUM") as pm1_ps):
             n_m = pm1b.tile([128, 8, TL], F32R, tag="n_m", name="n_m")
             for ch in range(NCH):
                 x2_ch = pm1.tile([128, 8, TC], F32R, tag="x_ch", name="x2_ch",
                                  bufs=1)
                 nc.sync.dma_start(
                     out=x2_ch,
                     in_=x2cm.rearrange("(k p) t -> p k t", p=128)[:, :, ts(ch, TC)])
                 nloc = layernorm(pm1, pm1_ps, x2_ch, gbm_sb, TC)
                 for kt in range(8):
                     nc.vector.tensor_copy(out=n_m[:, kt, ts(ch, TC)],
                                           in_=nloc[:, kt, :])
             for ft in range(32):
                 w_t = pm1.tile([128, 8, 128], F32R, tag="wstream", name="w_t",
                               bufs=3)
                 nc.sync.dma_start(out=w_t, in_=kpe(w1T)[:, :, ts(ft, 128)])
                 for ch in range(NCH):
                     ps1 = pm1_ps.tile([128, TC], F32, tag="ps_m1", name="ps1")
                     for kt in range(8):
                         nc.tensor.matmul(ps1, w_t[:, kt, :],
                                          n_m[:, kt, ts(ch, TC)],
                                          start=(kt == 0), stop=(kt == 7))
                     hft = pm1.tile([128, TC], F32R, tag="hft", name="hft")
                     nc.scalar.activation(hft, ps1, AF.Relu,
                                          bias=b1_sb[:, ft:ft + 1])
                     nc.sync.dma_start(out=hbuf[ts(ft, 128), ts(ch, TC)], in_=hft)

        if "M2" in PH:
         with (tc.tile_pool(name="pm2", bufs=3) as pm2,
              tc.tile_pool(name="pm2b", bufs=1) as pm2b,
              tc.tile_pool(name="pm2_ps", bufs=1, space="PSUM") as pm2_ps):
             w2_sb = pm2b.tile([128, 32, D], F32R, tag="w2_sb", name="w2_sb")
             nc.sync.dma_start(out=w2_sb, in_=kpe(w2T))
             for c2 in range(TL // MC2):
                 psD = []
                 for i in range(8):
                     pd = pm2_ps.tile([128, MC2], F32, tag=f"ps_m2_{i}",
                                      name=f"psD{i}", bufs=1)
                     psD.append(pd)
                 for ft in range(32):
                     hft2 = pm2.tile([128, MC2], F32R, tag="hstream", name="hft2")
                     nc.sync.dma_start(out=hft2,
                                       in_=hbuf[ts(ft, 128), ts(c2, MC2)])
                     for dt in range(8):
                         nc.tensor.matmul(
                             psD[dt],
                             w2_sb[:, ft, ts(dt, 128)], hft2,
                             start=(ft == 0), stop=(ft == 31))
                 x2_c2 = pm2.tile([128, 8, MC2], F32R, tag="x2_c2", name="x2_c2",
                                  bufs=2)
                 nc.sync.dma_start(
                     out=x2_c2,
                     in_=x2cm.rearrange("(k p) t -> p k t", p=128)[:, :, ts(c2, MC2)])
                 for dt in range(8):
                     yt_t = pm2.tile([128, MC2], F32R, tag="ytt", name="yt_t")
                     nc.vector.tensor_tensor(yt_t, psD[dt],
                                             x2_c2[:, dt, :], OP.add)
                     nc.vector.tensor_scalar_add(yt_t, yt_t, b2_sb[:, dt:dt + 1])
                     nc.sync.dma_start(out=yT[ts(dt, 128), ts(c2, MC2)].bitcast(F32R),
                                       in_=yt_t)
        cst_cm.__exit__(None, None, None)

    nc.finalize()
    in_names = ["xT", "wqcT", "wkcT", "wvcT", "wocT", "wqtT", "wktT", "wvtT",
                "wotT", "w1T", "w2T", "gb_c", "gb_t", "gb_m", "b1v", "b2v",
                "cq", "sq", "ck", "sk", "mkc", "mkt", "idm"]
    return nc, in_names


def _host_prep(inputs):
    """Build per-core in_maps from full inputs."""
    x = np.asarray(inputs["x"], np.float32)
    positions = np.asarray(inputs["positions"]).astype(np.int64)

    def T(a):
        return np.ascontiguousarray(np.asarray(a, np.float32).T)

    # temporal Q/K column permutation: [all evens (h-major, freq), all odds]
    perm = np.zeros(D, np.int64)
    for h in range(H_T):
        for i in range(32):
            perm[h * 32 + i] = h * 64 + 2 * i
            perm[512 + h * 32 + i] = h * 64 + 2 * i + 1
    wqtT = np.ascontiguousarray(T(inputs["Wq_t"])[:, perm])
    wktT = np.ascontiguousarray(T(inputs["Wk_t"])[:, perm])

    def gb(g, b):
        return np.ascontiguousarray(
            np.stack([np.asarray(g, np.float32), np.asarray(b, np.float32)],
                     axis=1))

    shared = {
        "wqcT": T(inputs["Wq_c"]), "wkcT": T(inputs["Wk_c"]),
        "wvcT": T(inputs["Wv_c"]), "wocT": T(inputs["Wo_c"]),
        "wqtT": wqtT, "wktT": wktT,
        "wvtT": T(inputs["Wv_t"]), "wotT": T(inputs["Wo_t"]),
        "w1T": T(inputs["W1"]), "w2T": T(inputs["W2"]),
        "gb_c": gb(inputs["g_c"], inputs["b_c"]),
        "gb_t": gb(inputs["g_t"], inputs["b_t"]),
        "gb_m": gb(inputs["g_m"], inputs["b_m"]),
        "b1v": np.asarray(inputs["b1"], np.float32).reshape(F_MLP, 1),
        "b2v": np.asarray(inputs["b2"], np.float32).reshape(D, 1),
    }
    # channel block-diag mask (tokens s-major, groups of 4)
    idx = np.arange(128)
    shared["mkc"] = (idx[:, None] // 4 == idx[None, :] // 4).astype(np.float32)
    # temporal causal masks for the 4 diagonal k-tiles of a 512 q-chunk
    mkt = np.zeros((4, 128, TC), np.float32)
    dq = np.arange(TC)
    dk = np.arange(128)
    for kt in range(4):
        mkt[kt] = (dq[None, :] >= kt * 128 + dk[:, None]).astype(np.float32)
    shared["mkt"] = mkt
    shared["idm"] = np.eye(128, dtype=np.float32)

    inv_freq = (10000.0 ** (-np.arange(32, dtype=np.float64) * 2 / HD_T))
    in_maps = []
    for i in range(N_CORES):
        m = dict(shared)
        xs = x[i * SB:(i + 1) * SB].reshape(TL, D)
        m["xT"] = np.ascontiguousarray(xs.T)
        pos = positions[i * SB:(i + 1) * SB].astype(np.float64)
        ang = pos[:, None] * inv_freq[None, :]          # [512, 32]
        cosT = np.cos(ang).T.astype(np.float32)         # [32, 512]
        sinT = np.sin(ang).T.astype(np.float32)
        c4 = np.tile(cosT, (4, 1))
        s4 = np.tile(sinT, (4, 1))
        m["cq"] = np.ascontiguousarray(c4 * 0.125)
        m["sq"] = np.ascontiguousarray(s4 * 0.125)
        m["ck"] = np.ascontiguousarray(c4)
        m["sk"] = np.ascontiguousarray(s4)
        in_maps.append(m)
    return in_maps


def _run(inputs, trace=False):
    from concourse.bass_utils import run_bass_kernel_spmd
    if "prog" not in _CACHE:
        _CACHE["prog"] = _build_program()
    nc, in_names = _CACHE["prog"]
    in_maps = _host_prep(inputs)
    for m in in_maps:
        for k in list(m.keys()):
            assert k in in_names, k
    res = run_bass_kernel_spmd(nc, in_maps, core_ids=list(range(N_CORES)),
                               trace=trace)
    out = np.zeros((S, C, D), np.float32)
    for i in range(N_CORES):
        yT = res.results[i]["yT"]                        # [1024, 2048] c-major
        yi = yT.T.reshape(C, SB, D)                      # [c, s, d]
        out[i * SB:(i + 1) * SB] = yi.transpose(1, 0, 2)
    return out, res


def kernel(**inputs) -> np.ndarray:
    out, _ = _run(inputs, trace=False)
    return out



# revision 31
# speedup vs baseline: 1.7947x; 1.7947x over previous
"""AxialTransformerBlock Trainium2 kernel (8 NeuronCores, SPMD + AllToAll).

Sharding: sequence-parallel over S (512 rows/core) for LN / channel attention /
MLP; head-parallel via one fused bf16 AllToAll for temporal causal attention
(8 (c,h) pairs per core over the full sequence), bf16 AllToAll back, then a
fused Wo_t + MLP pass per channel that keeps the residual in SBUF.

On-device layout: feature-major residual stream x^T [D, T], bf16 activations
with fp32 PSUM accumulation. Host pre-transposes/pre-tiles weights to bf16,
bakes RoPE cos/sin tables (even/odd de-interleave folded into the Wq_t/Wk_t
column permutation) and causal / channel block-diagonal masks.
"""

import contextlib

import numpy as np

N_CORES = 8
S, C, D = 4096, 4, 1024
SB = S // N_CORES          # 512 s-rows per core
TL = SB * C                # 2048 local tokens
H_T, HD_T = 16, 64
H_C, HD_C = 4, 256
F_MLP = 4 * D              # 4096
LN_EPS = 1e-5
TC = 512                   # token chunk
NCH = TL // TC             # 4
MC = 256                   # MLP second-gemm sub-chunk

_CACHE = {}


def _build_program():
    import concourse.bass as bass
    import concourse.bacc as bacc
    import concourse.tile as tile
    from concourse import mybir

    F32 = mybir.dt.float32
    BF16 = mybir.dt.bfloat16
    AF = mybir.ActivationFunctionType
    OP = mybir.AluOpType
    ts = bass.ts

    nc = bacc.Bacc("TRN2", target_bir_lowering=False, debug=False,
                   num_devices=N_CORES)

    def din(name, shape, dt=BF16):
        return nc.dram_tensor(name, list(shape), dt, kind="ExternalInput").ap()

    xT = din("xT", [D, TL])
    # phase-A weights, resident layout [128, kt, e_out]
    wqc = din("wqc", [128, 8, D])
    wkc = din("wkc", [128, 8, D])
    wvc = din("wvc", [128, 8, D])
    woc = din("woc", [128, 8, D])
    # phase-B projection weights, streamed layout [et, 128, kt, 128]
    wqt = din("wqt", [8, 128, 8, 128])
    wkt = din("wkt", [8, 128, 8, 128])
    wvt = din("wvt", [8, 128, 8, 128])
    # Wo_t streamed per output tile dt
    wot = din("wot", [8, 128, 8, 128])
    # MLP: W1 streamed per ft, W2 resident
    w1t = din("w1t", [32, 128, 8, 128])
    w2r = din("w2r", [128, 32, D])
    gb_c = din("gb_c", [D, 2], F32)   # col0 = g, col1 = b
    gb_t = din("gb_t", [D, 2], F32)
    gb_m = din("gb_m", [D, 2], F32)
    b1v = din("b1v", [F_MLP, 1], F32)
    b2v = din("b2v", [D, 1], F32)
    cq_d = din("cq", [128, 4, TC])
    sq_d = din("sq", [128, 4, TC])
    ck_d = din("ck", [128, 4, TC])
    sk_d = din("sk", [128, 4, TC])
    mkc_d = din("mkc4", [128, TC])
    mkt_d = din("mkt2", [2, 128, 1024])

    yT = nc.dram_tensor("yT", [D, TL], F32, kind="ExternalOutput").ap()

    import os
    dbg = os.environ.get("KDBG", "0") == "1"
    kindd = "ExternalOutput" if dbg else "Internal"
    PH = set(os.environ.get("KPHASES", "A,Bf,Ba,BM").split(","))
    x1cm = nc.dram_tensor("x1cm", [D, TL], BF16, kind=kindd).ap()
    # fused QKV all-to-all payload: per (dest, slot): sec0=Q[64,512],
    # sec1=K[64,512], sec2=V[512,64] (flat bytes)
    a2aI = nc.dram_tensor("a2aI", [8, 8, 3, 64 * TC], BF16).ap()
    a2aO = nc.dram_tensor("a2aO", [8, 8, 3, 64 * TC], BF16).ap()
    aAi = nc.dram_tensor("aAi", [8, 8, 64, TC], BF16).ap()
    aAo = nc.dram_tensor("aAo", [8, 8, 64, TC], BF16).ap()
    if dbg:
        a2aOd = nc.dram_tensor("a2aOd", [8, 8, 3, 64 * TC], BF16,
                               kind="ExternalOutput").ap()
        aAod = nc.dram_tensor("aAod", [8, 8, 64, TC], BF16,
                              kind="ExternalOutput").ap()
        xDbg = nc.dram_tensor("xDbg", [128, 8, TC], BF16,
                              kind="ExternalOutput").ap()
        nDbg = nc.dram_tensor("nDbg", [128, 8, TC], BF16,
                              kind="ExternalOutput").ap()
        qDbg = nc.dram_tensor("qDbg", [128, 8, TC], BF16,
                              kind="ExternalOutput").ap()
        kDbg = nc.dram_tensor("kDbg", [128, 8, TC], BF16,
                              kind="ExternalOutput").ap()
        vDbg = nc.dram_tensor("vDbg", [128, 4, D], BF16,
                              kind="ExternalOutput").ap()
        aDbg = nc.dram_tensor("aDbg", [128, 8, TC], BF16,
                              kind="ExternalOutput").ap()
        pDbg = nc.dram_tensor("pDbg", [128, TC], BF16,
                              kind="ExternalOutput").ap()
        rbDbg = nc.dram_tensor("rbDbg", [128, TC], BF16,
                               kind="ExternalOutput").ap()
        x2Dbg = nc.dram_tensor("x2Dbg", [128, 8, TC], BF16,
                               kind="ExternalOutput").ap()
        nmDbg = nc.dram_tensor("nmDbg", [128, 8, TC], BF16,
                               kind="ExternalOutput").ap()
        hDbg = nc.dram_tensor("hDbg", [128, 32, TC], BF16,
                              kind="ExternalOutput").ap()
        raDbg = nc.dram_tensor("raDbg", [128, 8, TC], BF16,
                               kind="ExternalOutput").ap()

    RG = [list(range(N_CORES))]

    with tile.TileContext(nc) as tc, \
            nc.allow_low_precision(reason="bf16 kernel; 2e-2 tolerance"):
        cst_cm = tc.tile_pool(name="cst", bufs=1)
        cst = cst_cm.__enter__()
        ones_mean = cst.tile([128, 1], BF16)      # 1/1024: stats matmuls
        nc.vector.memset(ones_mean, 1.0 / D)
        ones_one = cst.tile([128, 1], BF16)       # 1.0: channel-attn denom
        nc.vector.memset(ones_one, 1.0)
        eps1 = cst.tile([1, 1], F32)
        nc.vector.memset(eps1, LN_EPS)
        gbc_sb = cst.tile([128, 8, 2], F32)
        nc.sync.dma_start(out=gbc_sb, in_=gb_c.rearrange("(k p) two -> p k two", p=128))
        gbt_sb = cst.tile([128, 8, 2], F32)
        nc.sync.dma_start(out=gbt_sb, in_=gb_t.rearrange("(k p) two -> p k two", p=128))
        gbm_sb = cst.tile([128, 8, 2], F32)
        nc.sync.dma_start(out=gbm_sb, in_=gb_m.rearrange("(k p) two -> p k two", p=128))
        b1_sb = cst.tile([128, 32], F32)
        nc.sync.dma_start(out=b1_sb, in_=b1v.rearrange("(k p) one -> p (k one)", p=128))
        b2_sb = cst.tile([128, 8], F32)
        nc.sync.dma_start(out=b2_sb, in_=b2v.rearrange("(k p) one -> p (k one)", p=128))
        mkc_sb = cst.tile([128, TC], BF16)
        nc.sync.dma_start(out=mkc_sb, in_=mkc_d)
        mkt_sb = cst.tile([128, 2, 1024], BF16)
        nc.sync.dma_start(out=mkt_sb, in_=mkt_d.rearrange("b p q -> p b q"))

        def layernorm(pool, psum, x_ch, gb_sb):
            """x_ch [128, 8, TC] bf16 -> n_ch bf16 same shape."""
            st_s = psum.tile([128, TC], F32, tag="ps5", name="st_s", bufs=2)
            st_q = psum.tile([128, TC], F32, tag="ps5", name="st_q", bufs=2)
            for kt in range(8):
                nc.tensor.matmul(st_s[0:1, :], ones_mean, x_ch[:, kt, :],
                                 start=(kt == 0), stop=(kt == 7))
            for kt in range(8):
                xsq = pool.tile([128, TC], BF16, tag="ln_xsq", name="xsq",
                                bufs=2)
                nc.vector.tensor_tensor(xsq, x_ch[:, kt, :], x_ch[:, kt, :],
                                        OP.mult)
                nc.tensor.matmul(st_q[0:1, :], ones_mean, xsq,
                                 start=(kt == 0), stop=(kt == 7))
            mu2 = pool.tile([1, TC], F32, tag="ln_mu2", name="mu2")
            nc.scalar.activation(mu2, st_s[0:1, :], AF.Square)
            var = pool.tile([1, TC], F32, tag="ln_var", name="var")
            nc.vector.tensor_tensor(var, st_q[0:1, :], mu2, OP.subtract)
            sd = pool.tile([1, TC], F32, tag="ln_sd", name="sd")
            nc.scalar.activation(sd, var, AF.Sqrt, bias=eps1)
            rs = pool.tile([1, TC], BF16, tag="ln_rs", name="rs")
            nc.vector.reciprocal(rs, sd)
            bv = pool.tile([1, TC], BF16, tag="ln_bv", name="bv")
            nc.vector.tensor_tensor(bv, st_s[0:1, :], rs, OP.mult)
            ab = pool.tile([128, TC], BF16, tag="ln_ab", name="ab")
            nc.gpsimd.partition_broadcast(ab, rs)
            bb = pool.tile([128, TC], BF16, tag="ln_bb", name="bb")
            nc.gpsimd.partition_broadcast(bb, bv)
            n_ch = pool.tile([128, 8, TC], BF16, tag="ln_out", name="n_ch",
                             bufs=2)
            for kt in range(8):
                t1 = pool.tile([128, TC], BF16, tag="ln_t1", name="t1", bufs=2)
                nc.vector.tensor_tensor(t1, x_ch[:, kt, :], ab, OP.mult)
                nc.vector.tensor_tensor(t1, t1, bb, OP.subtract)
                nc.vector.tensor_scalar(n_ch[:, kt, :], t1,
                                        gb_sb[:, kt, 0:1], gb_sb[:, kt, 1:2],
                                        OP.mult, OP.add)
            return n_ch

        def proj_fmajor_res(psum, w_sb, n_ch, out_ch):
            """Resident weights [128, 8, D]: out_ch[:, et, :] feature-major."""
            for et in range(8):
                ps = psum.tile([128, TC], F32, tag="ps5", name="ps", bufs=2)
                for kt in range(8):
                    nc.tensor.matmul(ps, w_sb[:, kt, ts(et, 128)],
                                     n_ch[:, kt, :],
                                     start=(kt == 0), stop=(kt == 7))
                nc.scalar.activation(out_ch[:, et, :], ps, AF.Copy)

        def proj_fmajor_stream(pool, psum, wdram, n_ch, out_ch):
            """Streamed weights [et, 128, 8, 128]."""
            for et in range(8):
                w_t = pool.tile([128, 8, 128], BF16, tag="wstream", name="w_t",
                                bufs=3)
                nc.sync.dma_start(out=w_t, in_=wdram[et])
                ps = psum.tile([128, TC], F32, tag="ps5", name="ps", bufs=2)
                for kt in range(8):
                    nc.tensor.matmul(ps, w_t[:, kt, :], n_ch[:, kt, :],
                                     start=(kt == 0), stop=(kt == 7))
                nc.scalar.activation(out_ch[:, et, :], ps, AF.Copy)

        def proj_tmajor(pool, psum, wsrc, n_ch, out_ch, resident):
            """V token-major: out_ch [128, 4, 1024]."""
            for ec in range(2):
                psv = []
                for tt in range(4):
                    pv = psum.tile([128, TC], F32, tag="psv", name=f"pv{tt}",
                                   bufs=4)
                    psv.append(pv)
                for kt in range(8):
                    if resident:
                        w_mv = wsrc[:, kt, ts(ec, TC)]
                    else:
                        w_t = pool.tile([128, TC], BF16, tag="wstreamV",
                                        name="w_tv", bufs=3)
                        nc.sync.dma_start(
                            out=w_t.rearrange("p (a e) -> p a e", e=128),
                            in_=wsrc[4 * ec:4 * ec + 4, :, kt, :].rearrange(
                                "et p e -> p et e"))
                        w_mv = w_t
                    for tt in range(4):
                        nc.tensor.matmul(psv[tt], n_ch[:, kt, ts(tt, 128)],
                                         w_mv, start=(kt == 0), stop=(kt == 7))
                for tt in range(4):
                    nc.scalar.activation(out_ch[:, tt, ts(ec, TC)], psv[tt],
                                         AF.Copy)

        # ---------------- Phase A: channel attention ----------------
        if "A" in PH:
         with (tc.tile_pool(name="wa", bufs=1) as wa,
              tc.tile_pool(name="pa", bufs=2) as pa,
              tc.tile_pool(name="pa1", bufs=1) as pa1,
              tc.tile_pool(name="pa_ps", bufs=1, space="PSUM") as pa_ps):
             wqc_sb = wa.tile([128, 8, D], BF16, tag="wqc", name="wqc_sb")
             nc.sync.dma_start(out=wqc_sb, in_=wqc)
             wkc_sb = wa.tile([128, 8, D], BF16, tag="wkc", name="wkc_sb")
             nc.sync.dma_start(out=wkc_sb, in_=wkc)
             wvc_sb = wa.tile([128, 8, D], BF16, tag="wvc", name="wvc_sb")
             nc.sync.dma_start(out=wvc_sb, in_=wvc)
             woc_sb = wa.tile([128, 8, D], BF16, tag="woc", name="woc_sb")
             nc.sync.dma_start(out=woc_sb, in_=woc)
             for ch in range(NCH):
                 x_ch = pa.tile([128, 8, TC], BF16, tag="x_ch", name="x_ch",
                                bufs=2)
                 nc.sync.dma_start(
                     out=x_ch,
                     in_=xT.rearrange("(k p) t -> p k t", p=128)[:, :, ts(ch, TC)])
                 n_ch = layernorm(pa, pa_ps, x_ch, gbc_sb)
                 q_ch = pa1.tile([128, 8, TC], BF16, tag="q_ch", name="q_ch")
                 proj_fmajor_res(pa_ps, wqc_sb, n_ch, q_ch)
                 k_ch = pa1.tile([128, 8, TC], BF16, tag="k_ch", name="k_ch")
                 proj_fmajor_res(pa_ps, wkc_sb, n_ch, k_ch)
                 v_ch = pa1.tile([128, 4, D], BF16, tag="v_ch", name="v_ch")
                 proj_tmajor(pa, pa_ps, wvc_sb, n_ch, v_ch, resident=True)
                 if dbg and ch == 0:
                     nc.sync.dma_start(out=xDbg, in_=x_ch)
                     nc.sync.dma_start(out=nDbg, in_=n_ch)
                     nc.sync.dma_start(out=qDbg, in_=q_ch)
                     nc.sync.dma_start(out=kDbg, in_=k_ch)
                     nc.sync.dma_start(out=vDbg, in_=v_ch)

                 # attention: logits computed k-major [k, q], block-diag mask
                 aT_ch = pa1.tile([128, 8, TC], BF16, tag="aT_ch",
                                  name="aT_ch")
                 for h in range(H_C):
                     ps_l = pa_ps.tile([128, TC], F32, tag="psx", name="ps_l",
                                       bufs=2)
                     for qt in range(4):
                         for i, et in enumerate((2 * h, 2 * h + 1)):
                             nc.tensor.matmul(ps_l[:, ts(qt, 128)],
                                              k_ch[:, et, ts(qt, 128)],
                                              q_ch[:, et, ts(qt, 128)],
                                              start=(i == 0), stop=(i == 1))
                     pexp = pa.tile([128, TC], BF16, tag="pexp", name="pexp")
                     nc.scalar.activation(pexp, ps_l, AF.Exp, scale=1.0 / 16.0)
                     nc.vector.tensor_tensor(pexp, pexp, mkc_sb, OP.mult)
                     den = pa_ps.tile([128, TC], F32, tag="ps5", name="den",
                                      bufs=2)
                     nc.tensor.matmul(den[0:1, :], ones_one, pexp,
                                      start=True, stop=True)
                     rec = pa.tile([1, TC], BF16, tag="rec", name="rec")
                     nc.vector.reciprocal(rec, den[0:1, :])
                     rb = pa.tile([128, TC], BF16, tag="rb", name="rb")
                     nc.gpsimd.partition_broadcast(rb, rec)
                     if dbg and ch == 0 and h == 0:
                         nc.sync.dma_start(out=pDbg, in_=pexp)
                         nc.sync.dma_start(out=rbDbg, in_=rb)
                     for i, es in enumerate((2 * h, 2 * h + 1)):
                         ps_av = pa_ps.tile([128, TC], F32, tag="psv",
                                            name="ps_av", bufs=4)
                         for qt in range(4):
                             nc.tensor.matmul(ps_av[:, ts(qt, 128)],
                                              v_ch[:, qt, ts(es, 128)],
                                              pexp[:, ts(qt, 128)],
                                              start=True, stop=True)
                         # evict + normalize in one DVE op
                         nc.vector.tensor_tensor(aT_ch[:, es, :], ps_av, rb,
                                                 OP.mult)
                 # Wo + residual, write c-major bf16
                 if dbg and ch == 0:
                     nc.sync.dma_start(out=aDbg, in_=aT_ch)
                 x1w = pa.tile([128, 8, 4, 128], BF16, tag="x1w", name="x1w")
                 for dt in range(8):
                     ps_o = pa_ps.tile([128, TC], F32, tag="ps5", name="ps_o",
                                       bufs=2)
                     for et in range(8):
                         nc.tensor.matmul(ps_o, woc_sb[:, et, ts(dt, 128)],
                                          aT_ch[:, et, :],
                                          start=(et == 0), stop=(et == 7))
                     nc.vector.tensor_tensor(
                         x1w[:, dt].rearrange("p c s -> p s c"),
                         ps_o.rearrange("p (s c) -> p s c", c=4),
                         x_ch[:, dt, :].rearrange("p (s c) -> p s c", c=4),
                         OP.add)
                 for dt in range(8):
                     nc.sync.dma_start(
                         out=x1cm.rearrange("(k p) (c u) -> p k c u", p=128,
                                            c=4)[:, dt, :, ts(ch, 128)],
                         in_=x1w[:, dt])

        # ---------------- Phase B: temporal attention ----------------
        if PH & {"Bf", "Ba", "BM"}:
         with contextlib.ExitStack() as _bstk:
             if "Bf" in PH:
              with (tc.tile_pool(name="pb", bufs=2) as pb,
                   tc.tile_pool(name="pb1", bufs=1) as pb1,
                   tc.tile_pool(name="pb_ps", bufs=1, space="PSUM") as pb_ps):
                 cq_sb = pb1.tile([128, 4, TC], BF16, tag="cq", name="cq_sb")
                 nc.sync.dma_start(out=cq_sb, in_=cq_d)
                 sq_sb = pb1.tile([128, 4, TC], BF16, tag="sq", name="sq_sb")
                 nc.sync.dma_start(out=sq_sb, in_=sq_d)
                 ck_sb = pb1.tile([128, 4, TC], BF16, tag="ck", name="ck_sb")
                 nc.sync.dma_start(out=ck_sb, in_=ck_d)
                 sk_sb = pb1.tile([128, 4, TC], BF16, tag="sk", name="sk_sb")
                 nc.sync.dma_start(out=sk_sb, in_=sk_d)
                 for c in range(C):
                     x1_ch = pb.tile([128, 8, TC], BF16, tag="x_ch",
                                     name="x1_ch", bufs=2)
                     nc.sync.dma_start(
                         out=x1_ch,
                         in_=x1cm.rearrange("(k p) t -> p k t", p=128)[:, :, ts(c, TC)])
                     n_ch = layernorm(pb, pb_ps, x1_ch, gbt_sb)
                     q_ch = pb1.tile([128, 8, TC], BF16, tag="q_ch",
                                     name="q_ch", bufs=2)
                     proj_fmajor_stream(pb, pb_ps, wqt, n_ch, q_ch)
                     k_ch = pb1.tile([128, 8, TC], BF16, tag="k_ch",
                                     name="k_ch", bufs=2)
                     proj_fmajor_stream(pb, pb_ps, wkt, n_ch, k_ch)
                     v_ch = pb1.tile([128, 4, D], BF16, tag="v_ch",
                                     name="v_ch", bufs=2)
                     proj_tmajor(pb, pb_ps, wvt, n_ch, v_ch, resident=False)
                     # RoPE in place (ev tiles 0..3, od tiles 4..7)
                     for tgt, cos_sb, sin_sb in ((q_ch, cq_sb, sq_sb),
                                                 (k_ch, ck_sb, sk_sb)):
                         ev = tgt[:, 0:4, :]
                         od = tgt[:, 4:8, :]
                         t1 = pb.tile([128, 4, TC], BF16, tag="rp1", name="t1")
                         t2 = pb.tile([128, 4, TC], BF16, tag="rp2", name="t2")
                         t3 = pb.tile([128, 4, TC], BF16, tag="rp3", name="t3")
                         t4 = pb.tile([128, 4, TC], BF16, tag="rp4", name="t4")
                         nc.vector.tensor_tensor(t1, ev, cos_sb, OP.mult)
                         nc.vector.tensor_tensor(t2, ev, sin_sb, OP.mult)
                         nc.vector.tensor_tensor(t3, od, sin_sb, OP.mult)
                         nc.vector.tensor_tensor(t4, od, cos_sb, OP.mult)
                         nc.vector.tensor_tensor(ev, t1, t3, OP.subtract)
                         nc.vector.tensor_tensor(od, t2, t4, OP.add)
                     # scatter to the fused A2A buffer (6 DMAs per channel)
                     for j2 in range(2):
                         j = 2 * c + j2
                         for sec, src in ((0, q_ch), (1, k_ch)):
                             for f in range(2):
                                 for kt in range(2):
                                     nc.sync.dma_start(
                                         out=a2aI[j, :, sec].rearrange(
                                             "(kt pr) (f r q) -> kt f pr r q",
                                             kt=2, f=2, q=TC)[kt, f],
                                         in_=src[:, f * 4 + 2 * j2 + kt, :])
                         for tt in range(4):
                             nc.sync.dma_start(
                                 out=a2aI[j, :, 2].rearrange(
                                     "pl (tt p hd) -> tt p pl hd",
                                     p=128, hd=64)[tt],
                                 in_=v_ch[:, tt, ts(j2, TC)].rearrange(
                                     "p (pl hd) -> p pl hd", hd=64))
                 nc.gpsimd.collective_compute(
                     "AllToAll", OP.bypass, replica_groups=RG,
                     ins=[a2aI.opt()], outs=[a2aO.opt()])
                 if dbg:
                     nc.sync.dma_start(out=a2aOd, in_=a2aO)
             # W2 resident pool opens after Bf pools close; its DMA has no
             # dependency on the collective so it overlaps it
             wm = _bstk.enter_context(tc.tile_pool(name="wm", bufs=1))
             w2_sb = wm.tile([128, 32, D], BF16, tag="w2r", name="w2_sb")
             nc.sync.dma_start(out=w2_sb, in_=w2r)

             # flash attention per local pair over full S
             if "Ba" in PH:
              with (tc.tile_pool(name="pt", bufs=2) as pt,
                   tc.tile_pool(name="pt_ps", bufs=1, space="PSUM") as pt_ps):
                 for p in range(8):
                     kTp = pt.tile([64, 8, TC], BF16, tag="kTp", name="kTp",
                                   bufs=2)
                     nc.sync.dma_start(
                         out=kTp,
                         in_=a2aO[:, p, 1].rearrange("s (r q) -> r s q", q=TC))
                     vp = pt.tile([128, 32, 65], BF16, tag="vp", name="vp",
                                  bufs=2)
                     for k4 in range(4):
                         nc.sync.dma_start(
                             out=vp[:, :, 0:64].rearrange(
                                 "p (s k4) hd -> p s k4 hd", k4=4)[:, :, k4],
                             in_=a2aO[:, p, 2].rearrange(
                                 "s (k4 p hd) -> k4 p s hd", p=128, hd=64)[k4])
                     nc.vector.memset(vp[:, :, 64:65], 1.0)
                     qTp = pt.tile([64, 8, TC], BF16, tag="qTp", name="qTp",
                                   bufs=2)
                     nc.sync.dma_start(
                         out=qTp,
                         in_=a2aO[:, p, 0].rearrange("s (r q) -> r s q", q=TC))
                     aT_all = pt.tile([64, 8, TC], BF16, tag="aT_all",
                                      name="aT_all", bufs=2)
                     for qc in range(8):
                         ps_a = pt_ps.tile([128, TC], F32, tag="psa",
                                           name="ps_a", bufs=2)
                         nb = 2 * (qc + 1)
                         for b in range(nb):
                             ps2 = pt_ps.tile([128, 1024], F32,
                                              tag=f"pe{b % 2}", name="ps2",
                                              bufs=1)
                             for i in range(2):
                                 kt = 2 * b + i
                                 nc.tensor.matmul(
                                     ps2[:, ts(i, TC)],
                                     kTp[:, kt // 4, ts(kt % 4, 128)],
                                     qTp[:, qc, :], start=True, stop=True)
                             pexp = pt.tile([128, 1024], BF16, tag="pexp2",
                                            name="pexp", bufs=3)
                             nc.scalar.activation(pexp, ps2, AF.Exp)
                             if b >= nb - 2:
                                 nc.vector.tensor_tensor(
                                     pexp, pexp, mkt_sb[:, b - (nb - 2), :],
                                     OP.mult)
                             for i in range(2):
                                 kt = 2 * b + i
                                 nc.tensor.matmul(ps_a[0:65, :],
                                                  vp[:, kt, :],
                                                  pexp[:, ts(i, TC)],
                                                  start=(kt == 0),
                                                  stop=(kt == 4 * qc + 3))
                         rec1 = pt.tile([1, TC], BF16, tag="rec1", name="rec1")
                         nc.vector.reciprocal(rec1, ps_a[64:65, :])
                         rb1 = pt.tile([64, TC], BF16, tag="rb1", name="rb1")
                         nc.gpsimd.partition_broadcast(rb1, rec1)
                         nc.vector.tensor_tensor(aT_all[:, qc, :],
                                                 ps_a[0:64, :], rb1, OP.mult)
                     nc.sync.dma_start(
                         out=aAi[:, p].rearrange("s r q -> r s q"),
                         in_=aT_all)
                 nc.gpsimd.collective_compute(
                     "AllToAll", OP.bypass, replica_groups=RG,
                     ins=[aAi.opt()], outs=[aAo.opt()])
                 if dbg:
                     nc.sync.dma_start(out=aAod, in_=aAo)

             # ---- fused Wo_t + residual + MLP per channel ----
             if "BM" in PH:
              with (tc.tile_pool(name="pm", bufs=2) as pm,
                   tc.tile_pool(name="pm1", bufs=1) as pm1,
                   tc.tile_pool(name="pm_ps", bufs=1, space="PSUM") as pm_ps):
                 for c in range(C):
                     rhsA = pm.tile([128, 8, TC], BF16, tag="rhsA",
                                    name="rhsA", bufs=2)
                     nc.sync.dma_start(
                         out=rhsA,
                         in_=aAo[2 * c:2 * c + 2].rearrange(
                             "j2 (e2 lo) r q -> (lo r) (j2 e2) q", lo=2))
                     x1c = pm.tile([128, 8, TC], BF16, tag="x1c", name="x1c",
                                   bufs=2)
                     nc.sync.dma_start(
                         out=x1c,
                         in_=x1cm.rearrange("(k p) t -> p k t", p=128)[:, :, ts(c, TC)])
                     x2c = pm1.tile([128, 8, TC], BF16, tag="x2c", name="x2c",
                                    bufs=2)
                     for dt in range(8):
                         w_t = pm.tile([128, 8, 128], BF16, tag="wstream",
                                       name="w_t", bufs=3)
                         nc.sync.dma_start(out=w_t, in_=wot[dt])
                         ps_o = pm_ps.tile([128, TC], F32, tag="ps5",
                                           name="ps_o", bufs=2)
                         for et in range(8):
                             nc.tensor.matmul(ps_o, w_t[:, et, :],
                                              rhsA[:, et, :],
                                              start=(et == 0), stop=(et == 7))
                         nc.vector.tensor_tensor(x2c[:, dt, :], ps_o,
                                                 x1c[:, dt, :], OP.add)
                     if dbg and c == 0:
                         nc.sync.dma_start(out=raDbg, in_=rhsA)
                         nc.sync.dma_start(out=x2Dbg, in_=x2c)
                     n_m = layernorm(pm, pm_ps, x2c, gbm_sb)
                     h_m = pm1.tile([128, 32, TC], BF16, tag="h_m", name="h_m",
                                    bufs=1)
                     for ft in range(32):
                         w1_t = pm.tile([128, 8, 128], BF16, tag="wstream",
                                        name="w1_t", bufs=3)
                         nc.sync.dma_start(out=w1_t, in_=w1t[ft])
                         ps1 = pm_ps.tile([128, TC], F32, tag="ps5",
                                          name="ps1", bufs=2)
                         for kt in range(8):
                             nc.tensor.matmul(ps1, w1_t[:, kt, :],
                                              n_m[:, kt, :],
                                              start=(kt == 0), stop=(kt == 7))
                         nc.vector.tensor_scalar(h_m[:, ft, :], ps1,
                                                 b1_sb[:, ft:ft + 1], 0.0,
                                                 OP.add, OP.max)
                     if dbg and c == 0:
                         nc.sync.dma_start(out=nmDbg, in_=n_m)
                         nc.sync.dma_start(out=hDbg, in_=h_m)
                     # one full-width accumulation group per PSUM bank
                     for dh in range(2):
                         psD = []
                         for i in range(4):
                             pd = pm_ps.tile([128, TC], F32, tag=f"psD{i}",
                                             name=f"psD{i}", bufs=1)
                             psD.append(pd)
                         for ft in range(32):
                             for i in range(4):
                                 dt = dh * 4 + i
                                 nc.tensor.matmul(
                                     psD[i], w2_sb[:, ft, ts(dt, 128)],
                                     h_m[:, ft, :],
                                     start=(ft == 0), stop=(ft == 31))
                         y_c = pm.tile([128, 4, TC], F32, tag="y_c",
                                       name="y_c", bufs=1)
                         for i in range(4):
                             dt = dh * 4 + i
                             nc.vector.scalar_tensor_tensor(
                                 y_c[:, i, :], psD[i],
                                 b2_sb[:, dt:dt + 1],
                                 x2c[:, dt, :], OP.add, OP.add)
                         nc.sync.dma_start(
                             out=yT.rearrange("(k p) t -> p k t", p=128)[:, dh * 4:dh * 4 + 4, ts(c, TC)],
                             in_=y_c)
        cst_cm.__exit__(None, None, None)

    nc.finalize()
    in_names = ["xT", "wqc", "wkc", "wvc", "woc", "wqt", "wkt", "wvt",
                "wot", "w1t", "w2r", "gb_c", "gb_t", "gb_m", "b1v", "b2v",
                "cq", "sq", "ck", "sk", "mkc4", "mkt2"]
    return nc, in_names


def _host_prep(inputs):
    """Build per-core in_maps from full inputs."""
    import ml_dtypes
    BF = ml_dtypes.bfloat16
    x = np.asarray(inputs["x"], np.float32)
    positions = np.asarray(inputs["positions"]).astype(np.int64)

    def T(a):
        return np.ascontiguousarray(np.asarray(a, np.float32).T)

    def tile8(wT):          # [1024, E] -> [128, 8, E]
        return np.ascontiguousarray(
            wT.reshape(8, 128, -1).transpose(1, 0, 2))

    def tile_et(wT):        # [1024, 1024] -> [8(et), 128, 8(kt), 128]
        return np.ascontiguousarray(
            tile8(wT).reshape(128, 8, 8, 128).transpose(2, 0, 1, 3))

    # temporal Q/K column permutation: [all evens (h-major, freq), all odds]
    perm = np.zeros(D, np.int64)
    for h in range(H_T):
        for i in range(32):
            perm[h * 32 + i] = h * 64 + 2 * i
            perm[512 + h * 32 + i] = h * 64 + 2 * i + 1
    wqtT = np.ascontiguousarray(T(inputs["Wq_t"])[:, perm])
    wktT = np.ascontiguousarray(T(inputs["Wk_t"])[:, perm])

    def gb(g, b):
        return np.ascontiguousarray(
            np.stack([np.asarray(g, np.float32), np.asarray(b, np.float32)],
                     axis=1))

    w1T = T(inputs["W1"])            # [1024, 4096]
    w1_tiled = np.ascontiguousarray(
        tile8(w1T).reshape(128, 8, 32, 128).transpose(2, 0, 1, 3))
    w2T = T(inputs["W2"])            # [4096, 1024]
    w2_res = np.ascontiguousarray(w2T.reshape(32, 128, D).transpose(1, 0, 2))

    shared = {
        "wqc": tile8(T(inputs["Wq_c"])).astype(BF),
        "wkc": tile8(T(inputs["Wk_c"])).astype(BF),
        "wvc": tile8(T(inputs["Wv_c"])).astype(BF),
        "woc": tile8(T(inputs["Wo_c"])).astype(BF),
        "wqt": tile_et(wqtT).astype(BF),
        "wkt": tile_et(wktT).astype(BF),
        "wvt": tile_et(T(inputs["Wv_t"])).astype(BF),
        "wot": tile_et(T(inputs["Wo_t"])).astype(BF),
        "w1t": w1_tiled.astype(BF),
        "w2r": w2_res.astype(BF),
        "gb_c": gb(inputs["g_c"], inputs["b_c"]),
        "gb_t": gb(inputs["g_t"], inputs["b_t"]),
        "gb_m": gb(inputs["g_m"], inputs["b_m"]),
        "b1v": np.asarray(inputs["b1"], np.float32).reshape(F_MLP, 1),
        "b2v": np.asarray(inputs["b2"], np.float32).reshape(D, 1),
    }
    # channel block-diag mask (tokens s-major, groups of 4), tiled 4 qt
    idx = np.arange(128)
    mkc = (idx[:, None] // 4 == idx[None, :] // 4).astype(np.float32)
    shared["mkc4"] = np.tile(mkc, (1, 4)).astype(BF)
    # temporal causal masks: batches of two 128-row k-tiles
    dq = np.arange(TC)
    dk = np.arange(128)
    mkt2 = np.zeros((2, 128, 1024), np.float32)
    for b in range(2):
        for i in range(2):
            r = 2 * b + i
            mkt2[b][:, i * TC:(i + 1) * TC] = (
                dq[None, :] >= r * 128 + dk[:, None]).astype(np.float32)
    shared["mkt2"] = mkt2.astype(BF)

    inv_freq = (10000.0 ** (-np.arange(32, dtype=np.float64) * 2 / HD_T))
    in_maps = []
    for i in range(N_CORES):
        m = dict(shared)
        xs = x[i * SB:(i + 1) * SB].reshape(TL, D)
        m["xT"] = np.ascontiguousarray(xs.T).astype(BF)
        pos = positions[i * SB:(i + 1) * SB].astype(np.float64)
        ang = pos[:, None] * inv_freq[None, :]          # [512, 32]
        cosT = np.cos(ang).T.astype(np.float32)         # [32, 512]
        sinT = np.sin(ang).T.astype(np.float32)
        c4 = np.tile(cosT, (4, 1))                      # [128, 512]
        s4 = np.tile(sinT, (4, 1))
        m["cq"] = np.tile((c4 * 0.125)[:, None, :], (1, 4, 1)).astype(BF)
        m["sq"] = np.tile((s4 * 0.125)[:, None, :], (1, 4, 1)).astype(BF)
        m["ck"] = np.tile(c4[:, None, :], (1, 4, 1)).astype(BF)
        m["sk"] = np.tile(s4[:, None, :], (1, 4, 1)).astype(BF)
        in_maps.append(m)
    return in_maps


def _run(inputs, trace=False):
    from concourse.bass_utils import run_bass_kernel_spmd
    if "prog" not in _CACHE:
        _CACHE["prog"] = _build_program()
    nc, in_names = _CACHE["prog"]
    in_maps = _host_prep(inputs)
    for m in in_maps:
        for k in list(m.keys()):
            assert k in in_names, k
    res = run_bass_kernel_spmd(nc, in_maps, core_ids=list(range(N_CORES)),
                               trace=trace)
    out = np.zeros((S, C, D), np.float32)
    for i in range(N_CORES):
        yT = res.results[i]["yT"]                        # [1024, 2048] c-major
        yi = yT.T.reshape(C, SB, D)                      # [c, s, d]
        out[i * SB:(i + 1) * SB] = yi.transpose(1, 0, 2)
    return out, res


def kernel(**inputs) -> np.ndarray:
    out, _ = _run(inputs, trace=False)
    return out


# revision 35
# speedup vs baseline: 1.9101x; 1.0643x over previous
"""AxialTransformerBlock Trainium2 kernel (8 NeuronCores, SPMD + AllToAll).

Sharding: sequence-parallel over S (512 rows/core) for LN / channel attention /
MLP; head-parallel via one fused bf16 AllToAll for temporal causal attention
(8 (c,h) pairs per core over the full sequence), bf16 AllToAll back, then a
fused Wo_t + MLP pass per channel that keeps the residual in SBUF.

On-device layout: feature-major residual stream x^T [D, T], bf16 activations
with fp32 PSUM accumulation. Host pre-transposes/pre-tiles weights to bf16,
bakes RoPE cos/sin tables (even/odd de-interleave folded into the Wq_t/Wk_t
column permutation) and causal / channel block-diagonal masks.
"""

import contextlib

import numpy as np

N_CORES = 8
S, C, D = 4096, 4, 1024
SB = S // N_CORES          # 512 s-rows per core
TL = SB * C                # 2048 local tokens
H_T, HD_T = 16, 64
H_C, HD_C = 4, 256
F_MLP = 4 * D              # 4096
LN_EPS = 1e-5
TC = 512                   # token chunk
NCH = TL // TC             # 4
MC = 256                   # MLP second-gemm sub-chunk

_CACHE = {}


def _build_program():
    import concourse.bass as bass
    import concourse.bacc as bacc
    import concourse.tile as tile
    from concourse import mybir

    F32 = mybir.dt.float32
    BF16 = mybir.dt.bfloat16
    AF = mybir.ActivationFunctionType
    OP = mybir.AluOpType
    ts = bass.ts

    nc = bacc.Bacc("TRN2", target_bir_lowering=False, debug=False,
                   num_devices=N_CORES)

    def din(name, shape, dt=BF16):
        return nc.dram_tensor(name, list(shape), dt, kind="ExternalInput").ap()

    xT = din("xT", [D, TL])
    # phase-A weights, resident layout [128, kt, e_out]
    wqc = din("wqc", [128, 8, D])
    wkc = din("wkc", [128, 8, D])
    wvc = din("wvc", [128, 8, D])
    woc = din("woc", [128, 8, D])
    # phase-B projection weights, streamed layout [et, 128, kt, 128]
    wqt = din("wqt", [8, 128, 8, 128])
    wkt = din("wkt", [8, 128, 8, 128])
    wvt = din("wvt", [8, 128, 8, 128])
    # Wo_t streamed per output tile dt
    wot = din("wot", [8, 128, 8, 128])
    # MLP: W1 streamed per ft, W2 resident
    w1t = din("w1t", [32, 128, 8, 128])
    w2r = din("w2r", [128, 32, D])
    gb_c = din("gb_c", [D, 2], F32)   # col0 = g, col1 = b
    gb_t = din("gb_t", [D, 2], F32)
    gb_m = din("gb_m", [D, 2], F32)
    b1v = din("b1v", [F_MLP, 1], F32)
    b2v = din("b2v", [D, 1], F32)
    cq_d = din("cq", [128, 4, TC])
    sq_d = din("sq", [128, 4, TC])
    ck_d = din("ck", [128, 4, TC])
    sk_d = din("sk", [128, 4, TC])
    mkc_d = din("mkc4", [128, TC])
    mkt_d = din("mkt2", [2, 128, 1024])

    yT = nc.dram_tensor("yT", [D, TL], F32, kind="ExternalOutput").ap()

    import os
    dbg = os.environ.get("KDBG", "0") == "1"
    kindd = "ExternalOutput" if dbg else "Internal"
    PH = set(os.environ.get("KPHASES", "A,Bf,Ba,BM").split(","))
    x1cm = nc.dram_tensor("x1cm", [D, TL], BF16, kind=kindd).ap()
    # fused QKV all-to-all payload: per (dest, slot): sec0=Q[64,512],
    # sec1=K[64,512], sec2=V[512,64] (flat bytes)
    a2aIh = [nc.dram_tensor(f"a2aI{i}", [8, 4, 3, 64 * TC], BF16).ap()
             for i in range(2)]
    a2aOh = [nc.dram_tensor(f"a2aO{i}", [8, 4, 3, 64 * TC], BF16).ap()
             for i in range(2)]
    aAih = [nc.dram_tensor(f"aAi{i}", [8, 4, 64, TC], BF16).ap()
            for i in range(2)]
    aAoh = [nc.dram_tensor(f"aAo{i}", [8, 4, 64, TC], BF16).ap()
            for i in range(2)]
    if dbg:
        a2aOd = nc.dram_tensor("a2aOd", [8, 8, 3, 64 * TC], BF16,
                               kind="ExternalOutput").ap()
        aAod = nc.dram_tensor("aAod", [8, 8, 64, TC], BF16,
                              kind="ExternalOutput").ap()
        xDbg = nc.dram_tensor("xDbg", [128, 8, TC], BF16,
                              kind="ExternalOutput").ap()
        nDbg = nc.dram_tensor("nDbg", [128, 8, TC], BF16,
                              kind="ExternalOutput").ap()
        qDbg = nc.dram_tensor("qDbg", [128, 8, TC], BF16,
                              kind="ExternalOutput").ap()
        kDbg = nc.dram_tensor("kDbg", [128, 8, TC], BF16,
                              kind="ExternalOutput").ap()
        vDbg = nc.dram_tensor("vDbg", [128, 4, D], BF16,
                              kind="ExternalOutput").ap()
        aDbg = nc.dram_tensor("aDbg", [128, 8, TC], BF16,
                              kind="ExternalOutput").ap()
        pDbg = nc.dram_tensor("pDbg", [128, TC], BF16,
                              kind="ExternalOutput").ap()
        rbDbg = nc.dram_tensor("rbDbg", [128, TC], BF16,
                               kind="ExternalOutput").ap()
        x2Dbg = nc.dram_tensor("x2Dbg", [128, 8, TC], BF16,
                               kind="ExternalOutput").ap()
        nmDbg = nc.dram_tensor("nmDbg", [128, 8, TC], BF16,
                               kind="ExternalOutput").ap()
        hDbg = nc.dram_tensor("hDbg", [128, 32, TC], BF16,
                              kind="ExternalOutput").ap()
        raDbg = nc.dram_tensor("raDbg", [128, 8, TC], BF16,
                               kind="ExternalOutput").ap()

    RG = [list(range(N_CORES))]

    with tile.TileContext(nc) as tc, \
            nc.allow_low_precision(reason="bf16 kernel; 2e-2 tolerance"):
        cst_cm = tc.tile_pool(name="cst", bufs=1)
        cst = cst_cm.__enter__()
        ones_mean = cst.tile([128, 1], BF16)      # 1/1024: stats matmuls
        nc.vector.memset(ones_mean, 1.0 / D)
        ones_one = cst.tile([128, 1], BF16)       # 1.0: channel-attn denom
        nc.vector.memset(ones_one, 1.0)
        eps1 = cst.tile([1, 1], F32)
        nc.vector.memset(eps1, LN_EPS)
        gbc_sb = cst.tile([128, 8, 2], F32)
        nc.sync.dma_start(out=gbc_sb, in_=gb_c.rearrange("(k p) two -> p k two", p=128))
        gbt_sb = cst.tile([128, 8, 2], F32)
        nc.sync.dma_start(out=gbt_sb, in_=gb_t.rearrange("(k p) two -> p k two", p=128))
        gbm_sb = cst.tile([128, 8, 2], F32)
        nc.sync.dma_start(out=gbm_sb, in_=gb_m.rearrange("(k p) two -> p k two", p=128))
        b1_sb = cst.tile([128, 32], F32)
        nc.sync.dma_start(out=b1_sb, in_=b1v.rearrange("(k p) one -> p (k one)", p=128))
        b2_sb = cst.tile([128, 8], F32)
        nc.sync.dma_start(out=b2_sb, in_=b2v.rearrange("(k p) one -> p (k one)", p=128))
        mkc_sb = cst.tile([128, TC], BF16)
        nc.sync.dma_start(out=mkc_sb, in_=mkc_d)
        mkt_sb = cst.tile([128, 2, 1024], BF16)
        nc.sync.dma_start(out=mkt_sb, in_=mkt_d.rearrange("b p q -> p b q"))

        def layernorm(pool, psum, x_ch, gb_sb):
            """x_ch [128, 8, TC] bf16 -> n_ch bf16 same shape."""
            st_s = psum.tile([128, TC], F32, tag="ps5", name="st_s", bufs=2)
            st_q = psum.tile([128, TC], F32, tag="ps5", name="st_q", bufs=2)
            for kt in range(8):
                nc.tensor.matmul(st_s[0:1, :], ones_mean, x_ch[:, kt, :],
                                 start=(kt == 0), stop=(kt == 7))
            for kt in range(8):
                xsq = pool.tile([128, TC], BF16, tag="ln_xsq", name="xsq",
                                bufs=2)
                nc.vector.tensor_tensor(xsq, x_ch[:, kt, :], x_ch[:, kt, :],
                                        OP.mult)
                nc.tensor.matmul(st_q[0:1, :], ones_mean, xsq,
                                 start=(kt == 0), stop=(kt == 7))
            mu2 = pool.tile([1, TC], F32, tag="ln_mu2", name="mu2")
            nc.scalar.activation(mu2, st_s[0:1, :], AF.Square)
            var = pool.tile([1, TC], F32, tag="ln_var", name="var")
            nc.vector.tensor_tensor(var, st_q[0:1, :], mu2, OP.subtract)
            sd = pool.tile([1, TC], F32, tag="ln_sd", name="sd")
            nc.scalar.activation(sd, var, AF.Sqrt, bias=eps1)
            rs = pool.tile([1, TC], BF16, tag="ln_rs", name="rs")
            nc.vector.reciprocal(rs, sd)
            bv = pool.tile([1, TC], BF16, tag="ln_bv", name="bv")
            nc.vector.tensor_tensor(bv, st_s[0:1, :], rs, OP.mult)
            ab = pool.tile([128, TC], BF16, tag="ln_ab", name="ab")
            nc.gpsimd.partition_broadcast(ab, rs)
            bb = pool.tile([128, TC], BF16, tag="ln_bb", name="bb")
            nc.gpsimd.partition_broadcast(bb, bv)
            n_ch = pool.tile([128, 8, TC], BF16, tag="ln_out", name="n_ch",
                             bufs=2)
            for kt in range(8):
                t1 = pool.tile([128, TC], BF16, tag="ln_t1", name="t1", bufs=2)
                nc.vector.tensor_tensor(t1, x_ch[:, kt, :], ab, OP.mult)
                nc.vector.tensor_tensor(t1, t1, bb, OP.subtract)
                nc.vector.tensor_scalar(n_ch[:, kt, :], t1,
                                        gb_sb[:, kt, 0:1], gb_sb[:, kt, 1:2],
                                        OP.mult, OP.add)
            return n_ch

        def proj_fmajor_res(psum, w_sb, n_ch, out_ch):
            """Resident weights [128, 8, D]: out_ch[:, et, :] feature-major."""
            for et in range(8):
                ps = psum.tile([128, TC], F32, tag="ps5", name="ps", bufs=2)
                for kt in range(8):
                    nc.tensor.matmul(ps, w_sb[:, kt, ts(et, 128)],
                                     n_ch[:, kt, :],
                                     start=(kt == 0), stop=(kt == 7))
                nc.scalar.activation(out_ch[:, et, :], ps, AF.Copy)

        def proj_fmajor_stream(pool, psum, wdram, n_ch, out_ch):
            """Streamed weights [et, 128, 8, 128]."""
            for et in range(8):
                w_t = pool.tile([128, 8, 128], BF16, tag="wstream", name="w_t",
                                bufs=3)
                nc.sync.dma_start(out=w_t, in_=wdram[et])
                ps = psum.tile([128, TC], F32, tag="ps5", name="ps", bufs=2)
                for kt in range(8):
                    nc.tensor.matmul(ps, w_t[:, kt, :], n_ch[:, kt, :],
                                     start=(kt == 0), stop=(kt == 7))
                nc.scalar.activation(out_ch[:, et, :], ps, AF.Copy)

        def proj_tmajor(pool, psum, wsrc, n_ch, out_ch, resident):
            """V token-major: out_ch [128, 4, 1024]."""
            for ec in range(2):
                psv = []
                for tt in range(4):
                    pv = psum.tile([128, TC], F32, tag="psv", name=f"pv{tt}",
                                   bufs=4)
                    psv.append(pv)
                for kt in range(8):
                    if resident:
                        w_mv = wsrc[:, kt, ts(ec, TC)]
                    else:
                        w_t = pool.tile([128, TC], BF16, tag="wstreamV",
                                        name="w_tv", bufs=3)
                        nc.sync.dma_start(
                            out=w_t.rearrange("p (a e) -> p a e", e=128),
                            in_=wsrc[4 * ec:4 * ec + 4, :, kt, :].rearrange(
                                "et p e -> p et e"))
                        w_mv = w_t
                    for tt in range(4):
                        nc.tensor.matmul(psv[tt], n_ch[:, kt, ts(tt, 128)],
                                         w_mv, start=(kt == 0), stop=(kt == 7))
                for tt in range(4):
                    nc.scalar.activation(out_ch[:, tt, ts(ec, TC)], psv[tt],
                                         AF.Copy)

        # ---------------- Phase A: channel attention ----------------
        if "A" in PH:
         with (tc.tile_pool(name="wa", bufs=1) as wa,
              tc.tile_pool(name="pa", bufs=2) as pa,
              tc.tile_pool(name="pa1", bufs=1) as pa1,
              tc.tile_pool(name="pa_ps", bufs=1, space="PSUM") as pa_ps):
             x_pre = []
             for ch in range(2):
                 xt = pa.tile([128, 8, TC], BF16, tag="x_ch", name="x_ch",
                              bufs=2)
                 nc.sync.dma_start(
                     out=xt,
                     in_=xT.rearrange("(k p) t -> p k t", p=128)[:, :, ts(ch, TC)])
                 x_pre.append(xt)
             wqc_sb = wa.tile([128, 8, D], BF16, tag="wqc", name="wqc_sb")
             nc.sync.dma_start(out=wqc_sb, in_=wqc)
             wkc_sb = wa.tile([128, 8, D], BF16, tag="wkc", name="wkc_sb")
             nc.sync.dma_start(out=wkc_sb, in_=wkc)
             wvc_sb = wa.tile([128, 8, D], BF16, tag="wvc", name="wvc_sb")
             nc.sync.dma_start(out=wvc_sb, in_=wvc)
             woc_sb = wa.tile([128, 8, D], BF16, tag="woc", name="woc_sb")
             nc.sync.dma_start(out=woc_sb, in_=woc)
             for ch in range(NCH):
                 if ch < 2:
                     x_ch = x_pre[ch]
                 else:
                     x_ch = pa.tile([128, 8, TC], BF16, tag="x_ch",
                                    name="x_ch", bufs=2)
                     nc.sync.dma_start(
                         out=x_ch,
                         in_=xT.rearrange("(k p) t -> p k t", p=128)[:, :, ts(ch, TC)])
                 n_ch = layernorm(pa, pa_ps, x_ch, gbc_sb)
                 q_ch = pa1.tile([128, 8, TC], BF16, tag="q_ch", name="q_ch")
                 proj_fmajor_res(pa_ps, wqc_sb, n_ch, q_ch)
                 k_ch = pa1.tile([128, 8, TC], BF16, tag="k_ch", name="k_ch")
                 proj_fmajor_res(pa_ps, wkc_sb, n_ch, k_ch)
                 v_ch = pa1.tile([128, 4, D], BF16, tag="v_ch", name="v_ch")
                 proj_tmajor(pa, pa_ps, wvc_sb, n_ch, v_ch, resident=True)
                 if dbg and ch == 0:
                     nc.sync.dma_start(out=xDbg, in_=x_ch)
                     nc.sync.dma_start(out=nDbg, in_=n_ch)
                     nc.sync.dma_start(out=qDbg, in_=q_ch)
                     nc.sync.dma_start(out=kDbg, in_=k_ch)
                     nc.sync.dma_start(out=vDbg, in_=v_ch)

                 # attention: logits computed k-major [k, q], block-diag mask
                 aT_ch = pa1.tile([128, 8, TC], BF16, tag="aT_ch",
                                  name="aT_ch")
                 for h in range(H_C):
                     ps_l = pa_ps.tile([128, TC], F32, tag="psx", name="ps_l",
                                       bufs=2)
                     for qt in range(4):
                         for i, et in enumerate((2 * h, 2 * h + 1)):
                             nc.tensor.matmul(ps_l[:, ts(qt, 128)],
                                              k_ch[:, et, ts(qt, 128)],
                                              q_ch[:, et, ts(qt, 128)],
                                              start=(i == 0), stop=(i == 1))
                     pexp = pa.tile([128, TC], BF16, tag="pexp", name="pexp")
                     nc.scalar.activation(pexp, ps_l, AF.Exp, scale=1.0 / 16.0)
                     nc.vector.tensor_tensor(pexp, pexp, mkc_sb, OP.mult)
                     den = pa_ps.tile([128, TC], F32, tag="ps5", name="den",
                                      bufs=2)
                     nc.tensor.matmul(den[0:1, :], ones_one, pexp,
                                      start=True, stop=True)
                     rec = pa.tile([1, TC], BF16, tag="rec", name="rec")
                     nc.vector.reciprocal(rec, den[0:1, :])
                     rb = pa.tile([128, TC], BF16, tag="rb", name="rb")
                     nc.gpsimd.partition_broadcast(rb, rec)
                     if dbg and ch == 0 and h == 0:
                         nc.sync.dma_start(out=pDbg, in_=pexp)
                         nc.sync.dma_start(out=rbDbg, in_=rb)
                     for i, es in enumerate((2 * h, 2 * h + 1)):
                         ps_av = pa_ps.tile([128, TC], F32, tag="psv",
                                            name="ps_av", bufs=4)
                         for qt in range(4):
                             nc.tensor.matmul(ps_av[:, ts(qt, 128)],
                                              v_ch[:, qt, ts(es, 128)],
                                              pexp[:, ts(qt, 128)],
                                              start=True, stop=True)
                         # evict + normalize in one DVE op
                         nc.vector.tensor_tensor(aT_ch[:, es, :], ps_av, rb,
                                                 OP.mult)
                 # Wo + residual, write c-major bf16
                 if dbg and ch == 0:
                     nc.sync.dma_start(out=aDbg, in_=aT_ch)
                 x1w = pa.tile([128, 8, 4, 128], BF16, tag="x1w", name="x1w")
                 for dt in range(8):
                     ps_o = pa_ps.tile([128, TC], F32, tag="ps5", name="ps_o",
                                       bufs=2)
                     for et in range(8):
                         nc.tensor.matmul(ps_o, woc_sb[:, et, ts(dt, 128)],
                                          aT_ch[:, et, :],
                                          start=(et == 0), stop=(et == 7))
                     nc.vector.tensor_tensor(
                         x1w[:, dt].rearrange("p c s -> p s c"),
                         ps_o.rearrange("p (s c) -> p s c", c=4),
                         x_ch[:, dt, :].rearrange("p (s c) -> p s c", c=4),
                         OP.add)
                 for dt in range(8):
                     nc.sync.dma_start(
                         out=x1cm.rearrange("(k p) (c u) -> p k c u", p=128,
                                            c=4)[:, dt, :, ts(ch, 128)],
                         in_=x1w[:, dt])

        # ---------------- Phase B: temporal attention ----------------
        if PH & {"Bf", "Ba", "BM"}:
         with contextlib.ExitStack() as _bstk:
             if "Bf" in PH:
              with (tc.tile_pool(name="pb", bufs=2) as pb,
                   tc.tile_pool(name="pb1", bufs=1) as pb1,
                   tc.tile_pool(name="pb_ps", bufs=1, space="PSUM") as pb_ps):
                 cq_sb = pb1.tile([128, 4, TC], BF16, tag="cq", name="cq_sb")
                 nc.sync.dma_start(out=cq_sb, in_=cq_d)
                 sq_sb = pb1.tile([128, 4, TC], BF16, tag="sq", name="sq_sb")
                 nc.sync.dma_start(out=sq_sb, in_=sq_d)
                 ck_sb = pb1.tile([128, 4, TC], BF16, tag="ck", name="ck_sb")
                 nc.sync.dma_start(out=ck_sb, in_=ck_d)
                 sk_sb = pb1.tile([128, 4, TC], BF16, tag="sk", name="sk_sb")
                 nc.sync.dma_start(out=sk_sb, in_=sk_d)
                 for c in range(C):
                     x1_ch = pb.tile([128, 8, TC], BF16, tag="x_ch",
                                     name="x1_ch", bufs=2)
                     nc.sync.dma_start(
                         out=x1_ch,
                         in_=x1cm.rearrange("(k p) t -> p k t", p=128)[:, :, ts(c, TC)])
                     n_ch = layernorm(pb, pb_ps, x1_ch, gbt_sb)
                     q_ch = pb1.tile([128, 8, TC], BF16, tag="q_ch",
                                     name="q_ch", bufs=2)
                     proj_fmajor_stream(pb, pb_ps, wqt, n_ch, q_ch)
                     k_ch = pb1.tile([128, 8, TC], BF16, tag="k_ch",
                                     name="k_ch", bufs=2)
                     proj_fmajor_stream(pb, pb_ps, wkt, n_ch, k_ch)
                     v_ch = pb1.tile([128, 4, D], BF16, tag="v_ch",
                                     name="v_ch", bufs=2)
                     proj_tmajor(pb, pb_ps, wvt, n_ch, v_ch, resident=False)
                     # RoPE in place (ev tiles 0..3, od tiles 4..7)
                     for tgt, cos_sb, sin_sb in ((q_ch, cq_sb, sq_sb),
                                                 (k_ch, ck_sb, sk_sb)):
                         ev = tgt[:, 0:4, :]
                         od = tgt[:, 4:8, :]
                         t1 = pb.tile([128, 4, TC], BF16, tag="rp1", name="t1")
                         t2 = pb.tile([128, 4, TC], BF16, tag="rp2", name="t2")
                         t3 = pb.tile([128, 4, TC], BF16, tag="rp3", name="t3")
                         t4 = pb.tile([128, 4, TC], BF16, tag="rp4", name="t4")
                         nc.vector.tensor_tensor(t1, ev, cos_sb, OP.mult)
                         nc.vector.tensor_tensor(t2, ev, sin_sb, OP.mult)
                         nc.vector.tensor_tensor(t3, od, sin_sb, OP.mult)
                         nc.vector.tensor_tensor(t4, od, cos_sb, OP.mult)
                         nc.vector.tensor_tensor(ev, t1, t3, OP.subtract)
                         nc.vector.tensor_tensor(od, t2, t4, OP.add)
                     # scatter to the fused A2A buffer (6 DMAs per channel)
                     for j2 in range(2):
                         j = 2 * c + j2
                         for sec, src in ((0, q_ch), (1, k_ch)):
                             for f in range(2):
                                 for kt in range(2):
                                     nc.sync.dma_start(
                                         out=a2aIh[kt][j, :, sec].rearrange(
                                             "pr (f r q) -> f pr r q",
                                             f=2, q=TC)[f],
                                         in_=src[:, f * 4 + 2 * j2 + kt, :])
                         for tt in range(4):
                             for hf in range(2):
                                 nc.sync.dma_start(
                                     out=a2aIh[hf][j, :, 2].rearrange(
                                         "pl (tt p hd) -> tt p pl hd",
                                         p=128, hd=64)[tt],
                                     in_=v_ch[:, tt, j2 * TC + hf * 256:
                                              j2 * TC + hf * 256 + 256].rearrange(
                                         "p (pl hd) -> p pl hd", hd=64))
                 for hf in range(2):
                     nc.gpsimd.collective_compute(
                         "AllToAll", OP.bypass, replica_groups=RG,
                         ins=[a2aIh[hf].opt()], outs=[a2aOh[hf].opt()])
                 if dbg:
                     for hf in range(2):
                         nc.sync.dma_start(
                             out=a2aOd.rearrange(
                                 "s (hf pl) sec e -> hf s pl sec e", hf=2)[hf],
                             in_=a2aOh[hf])
             # W2 resident pool opens after Bf pools close; its DMA has no
             # dependency on the collective so it overlaps it
             wm = _bstk.enter_context(tc.tile_pool(name="wm", bufs=1))
             w2_sb = wm.tile([128, 32, D], BF16, tag="w2r", name="w2_sb")
             nc.sync.dma_start(out=w2_sb, in_=w2r)

             # flash attention per local pair over full S
             if "Ba" in PH:
              with (tc.tile_pool(name="pt", bufs=2) as pt,
                   tc.tile_pool(name="pt_ps", bufs=1, space="PSUM") as pt_ps):
                 for p in range(8):
                     abuf = a2aOh[p // 4]
                     pi = p % 4
                     kTp = pt.tile([64, 8, TC], BF16, tag="kTp", name="kTp",
                                   bufs=2)
                     nc.sync.dma_start(
                         out=kTp,
                         in_=abuf[:, pi, 1].rearrange("s (r q) -> r s q", q=TC))
                     vp = pt.tile([128, 32, 65], BF16, tag="vp", name="vp",
                                  bufs=2)
                     for k4 in range(4):
                         nc.sync.dma_start(
                             out=vp[:, :, 0:64].rearrange(
                                 "p (s k4) hd -> p s k4 hd", k4=4)[:, :, k4],
                             in_=abuf[:, pi, 2].rearrange(
                                 "s (k4 p hd) -> k4 p s hd", p=128, hd=64)[k4])
                     nc.vector.memset(vp[:, :, 64:65], 1.0)
                     qTp = pt.tile([64, 8, TC], BF16, tag="qTp", name="qTp",
                                   bufs=2)
                     nc.sync.dma_start(
                         out=qTp,
                         in_=abuf[:, pi, 0].rearrange("s (r q) -> r s q", q=TC))
                     aT_all = pt.tile([64, 8, TC], BF16, tag="aT_all",
                                      name="aT_all", bufs=2)
                     for qc in range(8):
                         ps_a = pt_ps.tile([128, TC], F32, tag="psa",
                                           name="ps_a", bufs=2)
                         nb = 2 * (qc + 1)
                         for b in range(nb):
                             ps2 = pt_ps.tile([128, 1024], F32,
                                              tag=f"pe{b % 2}", name="ps2",
                                              bufs=1)
                             for i in range(2):
                                 kt = 2 * b + i
                                 nc.tensor.matmul(
                                     ps2[:, ts(i, TC)],
                                     kTp[:, kt // 4, ts(kt % 4, 128)],
                                     qTp[:, qc, :], start=True, stop=True)
                             pexp = pt.tile([128, 1024], BF16, tag="pexp2",
                                            name="pexp", bufs=3)
                             nc.scalar.activation(pexp, ps2, AF.Exp)
                             if b >= nb - 2:
                                 nc.vector.tensor_tensor(
                                     pexp, pexp, mkt_sb[:, b - (nb - 2), :],
                                     OP.mult)
                             for i in range(2):
                                 kt = 2 * b + i
                                 nc.tensor.matmul(ps_a[0:65, :],
                                                  vp[:, kt, :],
                                                  pexp[:, ts(i, TC)],
                                                  start=(kt == 0),
                                                  stop=(kt == 4 * qc + 3))
                         rec1 = pt.tile([1, TC], BF16, tag="rec1", name="rec1")
                         nc.vector.reciprocal(rec1, ps_a[64:65, :])
                         rb1 = pt.tile([64, TC], BF16, tag="rb1", name="rb1")
                         nc.gpsimd.partition_broadcast(rb1, rec1)
                         nc.vector.tensor_tensor(aT_all[:, qc, :],
                                                 ps_a[0:64, :], rb1, OP.mult)
                     nc.sync.dma_start(
                         out=aAih[p // 4][:, p % 4].rearrange("s r q -> r s q"),
                         in_=aT_all)
                     if p % 4 == 3:
                         nc.gpsimd.collective_compute(
                             "AllToAll", OP.bypass, replica_groups=RG,
                             ins=[aAih[p // 4].opt()],
                             outs=[aAoh[p // 4].opt()])
                 if dbg:
                     for hf in range(2):
                         nc.sync.dma_start(
                             out=aAod.rearrange(
                                 "s (hf pl) r q -> hf s pl r q", hf=2)[hf],
                             in_=aAoh[hf])

             # ---- fused Wo_t + residual + MLP per channel ----
             if "BM" in PH:
              with (tc.tile_pool(name="pm", bufs=2) as pm,
                   tc.tile_pool(name="pm1", bufs=1) as pm1,
                   tc.tile_pool(name="pm_ps", bufs=1, space="PSUM") as pm_ps):
                 for c in range(C):
                     rhsAh = []
                     for hf in range(2):
                         rh = pm.tile([128, 4, TC], BF16, tag=f"rhsA{hf}",
                                      name=f"rhsA{hf}", bufs=2)
                         nc.sync.dma_start(
                             out=rh,
                             in_=aAoh[hf][2 * c:2 * c + 2].rearrange(
                                 "j2 (e2 lo) r q -> (lo r) (j2 e2) q", lo=2))
                         rhsAh.append(rh)
                     x1c = pm.tile([128, 8, TC], BF16, tag="x1c", name="x1c",
                                   bufs=2)
                     nc.sync.dma_start(
                         out=x1c,
                         in_=x1cm.rearrange("(k p) t -> p k t", p=128)[:, :, ts(c, TC)])
                     x2c = pm1.tile([128, 8, TC], BF16, tag="x2c", name="x2c",
                                    bufs=2)
                     for dt in range(8):
                         w_t = pm.tile([128, 8, 128], BF16, tag="wstream",
                                       name="w_t", bufs=3)
                         nc.sync.dma_start(out=w_t, in_=wot[dt])
                         ps_o = pm_ps.tile([128, TC], F32, tag="ps5",
                                           name="ps_o", bufs=2)
                         for et in range(8):
                             rsrc = rhsAh[(et % 4) // 2]
                             rcol = (et // 4) * 2 + (et % 2)
                             nc.tensor.matmul(ps_o, w_t[:, et, :],
                                              rsrc[:, rcol, :],
                                              start=(et == 0), stop=(et == 7))
                         nc.vector.tensor_tensor(x2c[:, dt, :], ps_o,
                                                 x1c[:, dt, :], OP.add)
                     if dbg and c == 0:
                         pass
                         nc.sync.dma_start(out=x2Dbg, in_=x2c)
                     n_m = layernorm(pm, pm_ps, x2c, gbm_sb)
                     h_m = pm1.tile([128, 32, TC], BF16, tag="h_m", name="h_m",
                                    bufs=1)
                     for ft in range(32):
                         w1_t = pm.tile([128, 8, 128], BF16, tag="wstream",
                                        name="w1_t", bufs=3)
                         nc.sync.dma_start(out=w1_t, in_=w1t[ft])
                         ps1 = pm_ps.tile([128, TC], F32, tag="ps5",
                                          name="ps1", bufs=2)
                         for kt in range(8):
                             nc.tensor.matmul(ps1, w1_t[:, kt, :],
                                              n_m[:, kt, :],
                                              start=(kt == 0), stop=(kt == 7))
                         nc.vector.tensor_scalar(h_m[:, ft, :], ps1,
                                                 b1_sb[:, ft:ft + 1], 0.0,
                                                 OP.add, OP.max)
                     if dbg and c == 0:
                         nc.sync.dma_start(out=nmDbg, in_=n_m)
                         nc.sync.dma_start(out=hDbg, in_=h_m)
                     # one full-width accumulation group per PSUM bank
                     for dh in range(2):
                         psD = []
                         for i in range(4):
                             pd = pm_ps.tile([128, TC], F32, tag=f"psD{i}",
                                             name=f"psD{i}", bufs=1)
                             psD.append(pd)
                         for ft in range(32):
                             for i in range(4):
                                 dt = dh * 4 + i
                                 nc.tensor.matmul(
                                     psD[i], w2_sb[:, ft, ts(dt, 128)],
                                     h_m[:, ft, :],
                                     start=(ft == 0), stop=(ft == 31))
                         y_c = pm.tile([128, 4, TC], F32, tag="y_c",
                                       name="y_c", bufs=1)
                         for i in range(4):
                             dt = dh * 4 + i
                             nc.vector.scalar_tensor_tensor(
                                 y_c[:, i, :], psD[i],
                                 b2_sb[:, dt:dt + 1],
                                 x2c[:, dt, :], OP.add, OP.add)
                         nc.sync.dma_start(
                             out=yT.rearrange("(k p) t -> p k t", p=128)[:, dh * 4:dh * 4 + 4, ts(c, TC)],
                             in_=y_c)
        cst_cm.__exit__(None, None, None)

    nc.finalize()
    in_names = ["xT", "wqc", "wkc", "wvc", "woc", "wqt", "wkt", "wvt",
                "wot", "w1t", "w2r", "gb_c", "gb_t", "gb_m", "b1v", "b2v",
                "cq", "sq", "ck", "sk", "mkc4", "mkt2"]
    return nc, in_names


def _host_prep(inputs):
    """Build per-core in_maps from full inputs."""
    import ml_dtypes
    BF = ml_dtypes.bfloat16
    x = np.asarray(inputs["x"], np.float32)
    positions = np.asarray(inputs["positions"]).astype(np.int64)

    def T(a):
        return np.ascontiguousarray(np.asarray(a, np.float32).T)

    def tile8(wT):          # [1024, E] -> [128, 8, E]
        return np.ascontiguousarray(
            wT.reshape(8, 128, -1).transpose(1, 0, 2))

    def tile_et(wT):        # [1024, 1024] -> [8(et), 128, 8(kt), 128]
        return np.ascontiguousarray(
            tile8(wT).reshape(128, 8, 8, 128).transpose(2, 0, 1, 3))

    # temporal Q/K column permutation: [all evens (h-major, freq), all odds]
    perm = np.zeros(D, np.int64)
    for h in range(H_T):
        for i in range(32):
            perm[h * 32 + i] = h * 64 + 2 * i
            perm[512 + h * 32 + i] = h * 64 + 2 * i + 1
    wqtT = np.ascontiguousarray(T(inputs["Wq_t"])[:, perm])
    wktT = np.ascontiguousarray(T(inputs["Wk_t"])[:, perm])

    def gb(g, b):
        return np.ascontiguousarray(
            np.stack([np.asarray(g, np.float32), np.asarray(b, np.float32)],
                     axis=1))

    w1T = T(inputs["W1"])            # [1024, 4096]
    w1_tiled = np.ascontiguousarray(
        tile8(w1T).reshape(128, 8, 32, 128).transpose(2, 0, 1, 3))
    w2T = T(inputs["W2"])            # [4096, 1024]
    w2_res = np.ascontiguousarray(w2T.reshape(32, 128, D).transpose(1, 0, 2))

    shared = {
        "wqc": tile8(T(inputs["Wq_c"])).astype(BF),
        "wkc": tile8(T(inputs["Wk_c"])).astype(BF),
        "wvc": tile8(T(inputs["Wv_c"])).astype(BF),
        "woc": tile8(T(inputs["Wo_c"])).astype(BF),
        "wqt": tile_et(wqtT).astype(BF),
        "wkt": tile_et(wktT).astype(BF),
        "wvt": tile_et(T(inputs["Wv_t"])).astype(BF),
        "wot": tile_et(T(inputs["Wo_t"])).astype(BF),
        "w1t": w1_tiled.astype(BF),
        "w2r": w2_res.astype(BF),
        "gb_c": gb(inputs["g_c"], inputs["b_c"]),
        "gb_t": gb(inputs["g_t"], inputs["b_t"]),
        "gb_m": gb(inputs["g_m"], inputs["b_m"]),
        "b1v": np.asarray(inputs["b1"], np.float32).reshape(F_MLP, 1),
        "b2v": np.asarray(inputs["b2"], np.float32).reshape(D, 1),
    }
    # channel block-diag mask (tokens s-major, groups of 4), tiled 4 qt
    idx = np.arange(128)
    mkc = (idx[:, None] // 4 == idx[None, :] // 4).astype(np.float32)
    shared["mkc4"] = np.tile(mkc, (1, 4)).astype(BF)
    # temporal causal masks: batches of two 128-row k-tiles
    dq = np.arange(TC)
    dk = np.arange(128)
    mkt2 = np.zeros((2, 128, 1024), np.float32)
    for b in range(2):
        for i in range(2):
            r = 2 * b + i
            mkt2[b][:, i * TC:(i + 1) * TC] = (
                dq[None, :] >= r * 128 + dk[:, None]).astype(np.float32)
    shared["mkt2"] = mkt2.astype(BF)

    inv_freq = (10000.0 ** (-np.arange(32, dtype=np.float64) * 2 / HD_T))
    in_maps = []
    for i in range(N_CORES):
        m = dict(shared)
        xs = x[i * SB:(i + 1) * SB].reshape(TL, D)
        m["xT"] = np.ascontiguousarray(xs.T).astype(BF)
        pos = positions[i * SB:(i + 1) * SB].astype(np.float64)
        ang = pos[:, None] * inv_freq[None, :]          # [512, 32]
        cosT = np.cos(ang).T.astype(np.float32)         # [32, 512]
        sinT = np.sin(ang).T.astype(np.float32)
        c4 = np.tile(cosT, (4, 1))                      # [128, 512]
        s4 = np.tile(sinT, (4, 1))
        m["cq"] = np.tile((c4 * 0.125)[:, None, :], (1, 4, 1)).astype(BF)
        m["sq"] = np.tile((s4 * 0.125)[:, None, :], (1, 4, 1)).astype(BF)
        m["ck"] = np.tile(c4[:, None, :], (1, 4, 1)).astype(BF)
        m["sk"] = np.tile(s4[:, None, :], (1, 4, 1)).astype(BF)
        in_maps.append(m)
    return in_maps


def _run(inputs, trace=False):
    from concourse.bass_utils import run_bass_kernel_spmd
    if "prog" not in _CACHE:
        _CACHE["prog"] = _build_program()
    nc, in_names = _CACHE["prog"]
    in_maps = _host_prep(inputs)
    for m in in_maps:
        for k in list(m.keys()):
            assert k in in_names, k
    res = run_bass_kernel_spmd(nc, in_maps, core_ids=list(range(N_CORES)),
                               trace=trace)
    out = np.zeros((S, C, D), np.float32)
    for i in range(N_CORES):
        yT = res.results[i]["yT"]                        # [1024, 2048] c-major
        yi = yT.T.reshape(C, SB, D)                      # [c, s, d]
        out[i * SB:(i + 1) * SB] = yi.transpose(1, 0, 2)
    return out, res


def kernel(**inputs) -> np.ndarray:
    out, _ = _run(inputs, trace=False)
    return out


# revision 39
# speedup vs baseline: 1.9429x; 1.0172x over previous
"""AxialTransformerBlock Trainium2 kernel (8 NeuronCores, SPMD + AllToAll).

Sharding: sequence-parallel over S (512 rows/core) for LN / channel attention /
MLP; head-parallel via one fused bf16 AllToAll for temporal causal attention
(8 (c,h) pairs per core over the full sequence), bf16 AllToAll back, then a
fused Wo_t + MLP pass per channel that keeps the residual in SBUF.

On-device layout: feature-major residual stream x^T [D, T], bf16 activations
with fp32 PSUM accumulation. Host pre-transposes/pre-tiles weights to bf16,
bakes RoPE cos/sin tables (even/odd de-interleave folded into the Wq_t/Wk_t
column permutation) and causal / channel block-diagonal masks.
"""

import contextlib

import numpy as np

N_CORES = 8
S, C, D = 4096, 4, 1024
SB = S // N_CORES          # 512 s-rows per core
TL = SB * C                # 2048 local tokens
H_T, HD_T = 16, 64
H_C, HD_C = 4, 256
F_MLP = 4 * D              # 4096
LN_EPS = 1e-5
TC = 512                   # token chunk
NCH = TL // TC             # 4
MC = 256                   # MLP second-gemm sub-chunk

_CACHE = {}


def _build_program():
    import concourse.bass as bass
    import concourse.bacc as bacc
    import concourse.tile as tile
    from concourse import mybir

    F32 = mybir.dt.float32
    BF16 = mybir.dt.bfloat16
    AF = mybir.ActivationFunctionType
    OP = mybir.AluOpType
    ts = bass.ts

    nc = bacc.Bacc("TRN2", target_bir_lowering=False, debug=False,
                   num_devices=N_CORES)

    def din(name, shape, dt=BF16):
        return nc.dram_tensor(name, list(shape), dt, kind="ExternalInput").ap()

    xT = din("xT", [D, TL])
    # phase-A weights, resident layout [128, kt, e_out]
    wqc = din("wqc", [128, 8, D])
    wkc = din("wkc", [128, 8, D])
    wvc = din("wvc", [128, 8, D])
    woc = din("woc", [128, 8, D])
    # phase-B projection weights, streamed layout [et, 128, kt, 128]
    wqt = din("wqt", [8, 128, 8, 128])
    wkt = din("wkt", [8, 128, 8, 128])
    wvt = din("wvt", [8, 128, 8, 128])
    # Wo_t streamed per output tile dt
    wot = din("wot", [8, 128, 8, 128])
    # MLP: W1 streamed per ft, W2 resident
    w1t = din("w1t", [32, 128, 8, 128])
    w2r = din("w2r", [128, 32, D])
    gb_c = din("gb_c", [D, 2], F32)   # col0 = g, col1 = b
    gb_t = din("gb_t", [D, 2], F32)
    gb_m = din("gb_m", [D, 2], F32)
    b1v = din("b1v", [F_MLP, 1], F32)
    b2v = din("b2v", [D, 1], F32)
    cq_d = din("cq", [128, 4, TC])
    sq_d = din("sq", [128, 4, TC])
    ck_d = din("ck", [128, 4, TC])
    sk_d = din("sk", [128, 4, TC])
    mkc_d = din("mkc4", [128, TC])
    mkt_d = din("mkt2", [2, 128, 1024])

    yT = nc.dram_tensor("yT", [D, TL], F32, kind="ExternalOutput").ap()

    import os
    dbg = os.environ.get("KDBG", "0") == "1"
    kindd = "ExternalOutput" if dbg else "Internal"
    PH = set(os.environ.get("KPHASES", "A,Bf,Ba,BM").split(","))
    x1cm = nc.dram_tensor("x1cm", [D, TL], BF16, kind=kindd).ap()
    # fused QKV all-to-all payload: per (dest, slot): sec0=Q[64,512],
    # sec1=K[64,512], sec2=V[512,64] (flat bytes)
    a2aIh = [nc.dram_tensor(f"a2aI{i}", [8, 4, 3, 64 * TC], BF16).ap()
             for i in range(2)]
    a2aOh = [nc.dram_tensor(f"a2aO{i}", [8, 4, 3, 64 * TC], BF16).ap()
             for i in range(2)]
    aAiP = [nc.dram_tensor(f"aAi{i}", [8, 64, TC], BF16).ap()
            for i in range(8)]
    aAoP = [nc.dram_tensor(f"aAo{i}", [8, 64, TC], BF16).ap()
            for i in range(8)]
    if dbg:
        a2aOd = nc.dram_tensor("a2aOd", [8, 8, 3, 64 * TC], BF16,
                               kind="ExternalOutput").ap()
        aAod = nc.dram_tensor("aAod", [8, 8, 64, TC], BF16,
                              kind="ExternalOutput").ap()
        xDbg = nc.dram_tensor("xDbg", [128, 8, TC], BF16,
                              kind="ExternalOutput").ap()
        nDbg = nc.dram_tensor("nDbg", [128, 8, TC], BF16,
                              kind="ExternalOutput").ap()
        qDbg = nc.dram_tensor("qDbg", [128, 8, TC], BF16,
                              kind="ExternalOutput").ap()
        kDbg = nc.dram_tensor("kDbg", [128, 8, TC], BF16,
                              kind="ExternalOutput").ap()
        vDbg = nc.dram_tensor("vDbg", [128, 4, D], BF16,
                              kind="ExternalOutput").ap()
        aDbg = nc.dram_tensor("aDbg", [128, 8, TC], BF16,
                              kind="ExternalOutput").ap()
        pDbg = nc.dram_tensor("pDbg", [128, TC], BF16,
                              kind="ExternalOutput").ap()
        rbDbg = nc.dram_tensor("rbDbg", [128, TC], BF16,
                               kind="ExternalOutput").ap()
        x2Dbg = nc.dram_tensor("x2Dbg", [128, 8, TC], BF16,
                               kind="ExternalOutput").ap()
        nmDbg = nc.dram_tensor("nmDbg", [128, 8, TC], BF16,
                               kind="ExternalOutput").ap()
        hDbg = nc.dram_tensor("hDbg", [128, 32, TC], BF16,
                              kind="ExternalOutput").ap()
        raDbg = nc.dram_tensor("raDbg", [128, 8, TC], BF16,
                               kind="ExternalOutput").ap()

    RG = [list(range(N_CORES))]

    with tile.TileContext(nc) as tc, \
            nc.allow_low_precision(reason="bf16 kernel; 2e-2 tolerance"):
        cst_cm = tc.tile_pool(name="cst", bufs=1)
        cst = cst_cm.__enter__()
        ones_mean = cst.tile([128, 1], BF16)      # 1/1024: stats matmuls
        nc.vector.memset(ones_mean, 1.0 / D)
        ones_one = cst.tile([128, 1], BF16)       # 1.0: channel-attn denom
        nc.vector.memset(ones_one, 1.0)
        eps1 = cst.tile([1, 1], F32)
        nc.vector.memset(eps1, LN_EPS)
        gbc_sb = cst.tile([128, 8, 2], F32)
        nc.sync.dma_start(out=gbc_sb, in_=gb_c.rearrange("(k p) two -> p k two", p=128))
        gbt_sb = cst.tile([128, 8, 2], F32)
        nc.sync.dma_start(out=gbt_sb, in_=gb_t.rearrange("(k p) two -> p k two", p=128))
        gbm_sb = cst.tile([128, 8, 2], F32)
        nc.sync.dma_start(out=gbm_sb, in_=gb_m.rearrange("(k p) two -> p k two", p=128))
        b1_sb = cst.tile([128, 32], F32)
        nc.sync.dma_start(out=b1_sb, in_=b1v.rearrange("(k p) one -> p (k one)", p=128))
        b2_sb = cst.tile([128, 8], F32)
        nc.sync.dma_start(out=b2_sb, in_=b2v.rearrange("(k p) one -> p (k one)", p=128))
        mkc_sb = cst.tile([128, TC], BF16)
        nc.sync.dma_start(out=mkc_sb, in_=mkc_d)
        mkt_sb = cst.tile([128, 2, 1024], BF16)
        nc.sync.dma_start(out=mkt_sb, in_=mkt_d.rearrange("b p q -> p b q"))

        def layernorm(pool, psum, x_ch, gb_sb):
            """x_ch [128, 8, TC] bf16 -> n_ch bf16 same shape."""
            st_s = psum.tile([128, TC], F32, tag="ps5", name="st_s", bufs=2)
            st_q = psum.tile([128, TC], F32, tag="ps5", name="st_q", bufs=2)
            for kt in range(8):
                nc.tensor.matmul(st_s[0:1, :], ones_mean, x_ch[:, kt, :],
                                 start=(kt == 0), stop=(kt == 7))
            for kt in range(8):
                xsq = pool.tile([128, TC], BF16, tag="ln_xsq", name="xsq",
                                bufs=2)
                nc.vector.tensor_tensor(xsq, x_ch[:, kt, :], x_ch[:, kt, :],
                                        OP.mult)
                nc.tensor.matmul(st_q[0:1, :], ones_mean, xsq,
                                 start=(kt == 0), stop=(kt == 7))
            mu2 = pool.tile([1, TC], F32, tag="ln_mu2", name="mu2")
            nc.scalar.activation(mu2, st_s[0:1, :], AF.Square)
            var = pool.tile([1, TC], F32, tag="ln_var", name="var")
            nc.vector.tensor_tensor(var, st_q[0:1, :], mu2, OP.subtract)
            sd = pool.tile([1, TC], F32, tag="ln_sd", name="sd")
            nc.scalar.activation(sd, var, AF.Sqrt, bias=eps1)
            rs = pool.tile([1, TC], BF16, tag="ln_rs", name="rs")
            nc.vector.reciprocal(rs, sd)
            bv = pool.tile([1, TC], BF16, tag="ln_bv", name="bv")
            nc.vector.tensor_tensor(bv, st_s[0:1, :], rs, OP.mult)
            ab = pool.tile([128, TC], BF16, tag="ln_ab", name="ab")
            nc.gpsimd.partition_broadcast(ab, rs)
            bb = pool.tile([128, TC], BF16, tag="ln_bb", name="bb")
            nc.gpsimd.partition_broadcast(bb, bv)
            n_ch = pool.tile([128, 8, TC], BF16, tag="ln_out", name="n_ch",
                             bufs=2)
            for kt in range(8):
                t1 = pool.tile([128, TC], BF16, tag="ln_t1", name="t1", bufs=2)
                nc.vector.tensor_tensor(t1, x_ch[:, kt, :], ab, OP.mult)
                nc.vector.tensor_tensor(t1, t1, bb, OP.subtract)
                nc.vector.tensor_scalar(n_ch[:, kt, :], t1,
                                        gb_sb[:, kt, 0:1], gb_sb[:, kt, 1:2],
                                        OP.mult, OP.add)
            return n_ch

        def proj_fmajor_res(psum, w_sb, n_ch, out_ch):
            """Resident weights [128, 8, D]: out_ch[:, et, :] feature-major."""
            for et in range(8):
                ps = psum.tile([128, TC], F32, tag="ps5", name="ps", bufs=2)
                for kt in range(8):
                    nc.tensor.matmul(ps, w_sb[:, kt, ts(et, 128)],
                                     n_ch[:, kt, :],
                                     start=(kt == 0), stop=(kt == 7))
                nc.scalar.activation(out_ch[:, et, :], ps, AF.Copy)

        def proj_fmajor_stream(pool, psum, wdram, n_ch, out_ch):
            """Streamed weights [et, 128, 8, 128]."""
            for et in range(8):
                w_t = pool.tile([128, 8, 128], BF16, tag="wstream", name="w_t",
                                bufs=3)
                nc.sync.dma_start(out=w_t, in_=wdram[et])
                ps = psum.tile([128, TC], F32, tag="ps5", name="ps", bufs=2)
                for kt in range(8):
                    nc.tensor.matmul(ps, w_t[:, kt, :], n_ch[:, kt, :],
                                     start=(kt == 0), stop=(kt == 7))
                nc.scalar.activation(out_ch[:, et, :], ps, AF.Copy)

        def proj_tmajor(pool, psum, wsrc, n_ch, out_ch, resident):
            """V token-major: out_ch [128, 4, 1024]."""
            for ec in range(2):
                psv = []
                for tt in range(4):
                    pv = psum.tile([128, TC], F32, tag="psv", name=f"pv{tt}",
                                   bufs=4)
                    psv.append(pv)
                for kt in range(8):
                    if resident:
                        w_mv = wsrc[:, kt, ts(ec, TC)]
                    else:
                        w_t = pool.tile([128, TC], BF16, tag="wstreamV",
                                        name="w_tv", bufs=3)
                        nc.sync.dma_start(
                            out=w_t.rearrange("p (a e) -> p a e", e=128),
                            in_=wsrc[4 * ec:4 * ec + 4, :, kt, :].rearrange(
                                "et p e -> p et e"))
                        w_mv = w_t
                    for tt in range(4):
                        nc.tensor.matmul(psv[tt], n_ch[:, kt, ts(tt, 128)],
                                         w_mv, start=(kt == 0), stop=(kt == 7))
                for tt in range(4):
                    nc.scalar.activation(out_ch[:, tt, ts(ec, TC)], psv[tt],
                                         AF.Copy)

        # ---------------- Phase A: channel attention ----------------
        if "A" in PH:
         with (tc.tile_pool(name="wa", bufs=1) as wa,
              tc.tile_pool(name="pa", bufs=2) as pa,
              tc.tile_pool(name="pa1", bufs=1) as pa1,
              tc.tile_pool(name="pa_ps", bufs=1, space="PSUM") as pa_ps):
             x_pre = []
             for ch in range(2):
                 xt = pa.tile([128, 8, TC], BF16, tag="x_ch", name="x_ch",
                              bufs=2)
                 nc.sync.dma_start(
                     out=xt,
                     in_=xT.rearrange("(k p) t -> p k t", p=128)[:, :, ts(ch, TC)])
                 x_pre.append(xt)
             wqc_sb = wa.tile([128, 8, D], BF16, tag="wqc", name="wqc_sb")
             nc.sync.dma_start(out=wqc_sb, in_=wqc)
             wkc_sb = wa.tile([128, 8, D], BF16, tag="wkc", name="wkc_sb")
             nc.sync.dma_start(out=wkc_sb, in_=wkc)
             wvc_sb = wa.tile([128, 8, D], BF16, tag="wvc", name="wvc_sb")
             nc.sync.dma_start(out=wvc_sb, in_=wvc)
             woc_sb = wa.tile([128, 8, D], BF16, tag="woc", name="woc_sb")
             nc.sync.dma_start(out=woc_sb, in_=woc)
             for ch in range(NCH):
                 if ch < 2:
                     x_ch = x_pre[ch]
                 else:
                     x_ch = pa.tile([128, 8, TC], BF16, tag="x_ch",
                                    name="x_ch", bufs=2)
                     nc.sync.dma_start(
                         out=x_ch,
                         in_=xT.rearrange("(k p) t -> p k t", p=128)[:, :, ts(ch, TC)])
                 n_ch = layernorm(pa, pa_ps, x_ch, gbc_sb)
                 q_ch = pa1.tile([128, 8, TC], BF16, tag="q_ch", name="q_ch")
                 proj_fmajor_res(pa_ps, wqc_sb, n_ch, q_ch)
                 k_ch = pa1.tile([128, 8, TC], BF16, tag="k_ch", name="k_ch")
                 proj_fmajor_res(pa_ps, wkc_sb, n_ch, k_ch)
                 v_ch = pa1.tile([128, 4, D], BF16, tag="v_ch", name="v_ch")
                 proj_tmajor(pa, pa_ps, wvc_sb, n_ch, v_ch, resident=True)
                 if dbg and ch == 0:
                     nc.sync.dma_start(out=xDbg, in_=x_ch)
                     nc.sync.dma_start(out=nDbg, in_=n_ch)
                     nc.sync.dma_start(out=qDbg, in_=q_ch)
                     nc.sync.dma_start(out=kDbg, in_=k_ch)
                     nc.sync.dma_start(out=vDbg, in_=v_ch)

                 # attention: logits computed k-major [k, q], block-diag mask
                 aT_ch = pa1.tile([128, 8, TC], BF16, tag="aT_ch",
                                  name="aT_ch")
                 for h in range(H_C):
                     ps_l = pa_ps.tile([128, TC], F32, tag="psx", name="ps_l",
                                       bufs=2)
                     for qt in range(4):
                         for i, et in enumerate((2 * h, 2 * h + 1)):
                             nc.tensor.matmul(ps_l[:, ts(qt, 128)],
                                              k_ch[:, et, ts(qt, 128)],
                                              q_ch[:, et, ts(qt, 128)],
                                              start=(i == 0), stop=(i == 1))
                     pexp = pa.tile([128, TC], BF16, tag="pexp", name="pexp")
                     nc.scalar.activation(pexp, ps_l, AF.Exp, scale=1.0 / 16.0)
                     nc.vector.tensor_tensor(pexp, pexp, mkc_sb, OP.mult)
                     den = pa_ps.tile([128, TC], F32, tag="ps5", name="den",
                                      bufs=2)
                     nc.tensor.matmul(den[0:1, :], ones_one, pexp,
                                      start=True, stop=True)
                     rec = pa.tile([1, TC], BF16, tag="rec", name="rec")
                     nc.vector.reciprocal(rec, den[0:1, :])
                     rb = pa.tile([128, TC], BF16, tag="rb", name="rb")
                     nc.gpsimd.partition_broadcast(rb, rec)
                     if dbg and ch == 0 and h == 0:
                         nc.sync.dma_start(out=pDbg, in_=pexp)
                         nc.sync.dma_start(out=rbDbg, in_=rb)
                     for i, es in enumerate((2 * h, 2 * h + 1)):
                         ps_av = pa_ps.tile([128, TC], F32, tag="psv",
                                            name="ps_av", bufs=4)
                         for qt in range(4):
                             nc.tensor.matmul(ps_av[:, ts(qt, 128)],
                                              v_ch[:, qt, ts(es, 128)],
                                              pexp[:, ts(qt, 128)],
                                              start=True, stop=True)
                         # evict + normalize in one DVE op
                         nc.vector.tensor_tensor(aT_ch[:, es, :], ps_av, rb,
                                                 OP.mult)
                 # Wo + residual, write c-major bf16
                 if dbg and ch == 0:
                     nc.sync.dma_start(out=aDbg, in_=aT_ch)
                 x1w = pa.tile([128, 8, 4, 128], BF16, tag="x1w", name="x1w")
                 for dt in range(8):
                     ps_o = pa_ps.tile([128, TC], F32, tag="ps5", name="ps_o",
                                       bufs=2)
                     for et in range(8):
                         nc.tensor.matmul(ps_o, woc_sb[:, et, ts(dt, 128)],
                                          aT_ch[:, et, :],
                                          start=(et == 0), stop=(et == 7))
                     nc.vector.tensor_tensor(
                         x1w[:, dt].rearrange("p c s -> p s c"),
                         ps_o.rearrange("p (s c) -> p s c", c=4),
                         x_ch[:, dt, :].rearrange("p (s c) -> p s c", c=4),
                         OP.add)
                 for dt in range(8):
                     nc.sync.dma_start(
                         out=x1cm.rearrange("(k p) (c u) -> p k c u", p=128,
                                            c=4)[:, dt, :, ts(ch, 128)],
                         in_=x1w[:, dt])

        # ---------------- Phase B: temporal attention ----------------
        if PH & {"Bf", "Ba", "BM"}:
         with contextlib.ExitStack() as _bstk:
             if "Bf" in PH:
              with (tc.tile_pool(name="pb", bufs=2) as pb,
                   tc.tile_pool(name="pb1", bufs=1) as pb1,
                   tc.tile_pool(name="pb_ps", bufs=1, space="PSUM") as pb_ps):
                 cq_sb = pb1.tile([128, 4, TC], BF16, tag="cq", name="cq_sb")
                 nc.sync.dma_start(out=cq_sb, in_=cq_d)
                 sq_sb = pb1.tile([128, 4, TC], BF16, tag="sq", name="sq_sb")
                 nc.sync.dma_start(out=sq_sb, in_=sq_d)
                 ck_sb = pb1.tile([128, 4, TC], BF16, tag="ck", name="ck_sb")
                 nc.sync.dma_start(out=ck_sb, in_=ck_d)
                 sk_sb = pb1.tile([128, 4, TC], BF16, tag="sk", name="sk_sb")
                 nc.sync.dma_start(out=sk_sb, in_=sk_d)
                 x1_pre = pb.tile([128, 8, TC], BF16, tag="x_ch",
                                  name="x1_ch", bufs=2)
                 nc.sync.dma_start(
                     out=x1_pre,
                     in_=x1cm.rearrange("(k p) t -> p k t", p=128)[:, :, ts(0, TC)])
                 for c in range(C):
                     x1_ch = x1_pre
                     if c + 1 < C:
                         x1_pre = pb.tile([128, 8, TC], BF16, tag="x_ch",
                                          name="x1_ch", bufs=2)
                         nc.sync.dma_start(
                             out=x1_pre,
                             in_=x1cm.rearrange("(k p) t -> p k t", p=128)[:, :, ts(c + 1, TC)])
                     n_ch = layernorm(pb, pb_ps, x1_ch, gbt_sb)
                     q_ch = pb1.tile([128, 8, TC], BF16, tag="q_ch",
                                     name="q_ch", bufs=2)
                     proj_fmajor_stream(pb, pb_ps, wqt, n_ch, q_ch)
                     k_ch = pb1.tile([128, 8, TC], BF16, tag="k_ch",
                                     name="k_ch", bufs=2)
                     proj_fmajor_stream(pb, pb_ps, wkt, n_ch, k_ch)
                     v_ch = pb1.tile([128, 4, D], BF16, tag="v_ch",
                                     name="v_ch", bufs=2)
                     proj_tmajor(pb, pb_ps, wvt, n_ch, v_ch, resident=False)
                     # RoPE in place (ev tiles 0..3, od tiles 4..7)
                     for tgt, cos_sb, sin_sb in ((q_ch, cq_sb, sq_sb),
                                                 (k_ch, ck_sb, sk_sb)):
                         ev = tgt[:, 0:4, :]
                         od = tgt[:, 4:8, :]
                         t1 = pb.tile([128, 4, TC], BF16, tag="rp1", name="t1")
                         t2 = pb.tile([128, 4, TC], BF16, tag="rp2", name="t2")
                         t3 = pb.tile([128, 4, TC], BF16, tag="rp3", name="t3")
                         t4 = pb.tile([128, 4, TC], BF16, tag="rp4", name="t4")
                         nc.vector.tensor_tensor(t1, ev, cos_sb, OP.mult)
                         nc.vector.tensor_tensor(t2, ev, sin_sb, OP.mult)
                         nc.vector.tensor_tensor(t3, od, sin_sb, OP.mult)
                         nc.vector.tensor_tensor(t4, od, cos_sb, OP.mult)
                         nc.vector.tensor_tensor(ev, t1, t3, OP.subtract)
                         nc.vector.tensor_tensor(od, t2, t4, OP.add)
                     # scatter to the fused A2A buffer (6 DMAs per channel)
                     for j2 in range(2):
                         j = 2 * c + j2
                         for sec, src in ((0, q_ch), (1, k_ch)):
                             for f in range(2):
                                 for kt in range(2):
                                     nc.sync.dma_start(
                                         out=a2aIh[kt][j, :, sec].rearrange(
                                             "pr (f r q) -> f pr r q",
                                             f=2, q=TC)[f],
                                         in_=src[:, f * 4 + 2 * j2 + kt, :])
                         for tt in range(4):
                             for hf in range(2):
                                 nc.scalar.dma_start(
                                     out=a2aIh[hf][j, :, 2].rearrange(
                                         "pl (tt p hd) -> tt p pl hd",
                                         p=128, hd=64)[tt],
                                     in_=v_ch[:, tt, j2 * TC + hf * 256:
                                              j2 * TC + hf * 256 + 256].rearrange(
                                         "p (pl hd) -> p pl hd", hd=64))
                 for hf in range(2):
                     nc.gpsimd.collective_compute(
                         "AllToAll", OP.bypass, replica_groups=RG,
                         ins=[a2aIh[hf].opt()], outs=[a2aOh[hf].opt()])
                 if dbg:
                     for hf in range(2):
                         nc.sync.dma_start(
                             out=a2aOd.rearrange(
                                 "s (hf pl) sec e -> hf s pl sec e", hf=2)[hf],
                             in_=a2aOh[hf])
             # W2 resident pool opens after Bf pools close; its DMA has no
             # dependency on the collective so it overlaps it
             wm = _bstk.enter_context(tc.tile_pool(name="wm", bufs=1))
             w2_sb = wm.tile([128, 32, D], BF16, tag="w2r", name="w2_sb")
             nc.sync.dma_start(out=w2_sb, in_=w2r)

             # flash attention per local pair over full S
             if "Ba" in PH:
              with (tc.tile_pool(name="pt", bufs=2) as pt,
                   tc.tile_pool(name="pt_ps", bufs=1, space="PSUM") as pt_ps):
                 for p in range(8):
                     abuf = a2aOh[p // 4]
                     pi = p % 4
                     kTp = pt.tile([64, 8, TC], BF16, tag="kTp", name="kTp",
                                   bufs=2)
                     nc.sync.dma_start(
                         out=kTp,
                         in_=abuf[:, pi, 1].rearrange("s (r q) -> r s q", q=TC))
                     vp = pt.tile([128, 32, 65], BF16, tag="vp", name="vp",
                                  bufs=2)
                     for k4 in range(4):
                         nc.sync.dma_start(
                             out=vp[:, :, 0:64].rearrange(
                                 "p (s k4) hd -> p s k4 hd", k4=4)[:, :, k4],
                             in_=abuf[:, pi, 2].rearrange(
                                 "s (k4 p hd) -> k4 p s hd", p=128, hd=64)[k4])
                     nc.vector.memset(vp[:, :, 64:65], 1.0)
                     qTp = pt.tile([64, 8, TC], BF16, tag="qTp", name="qTp",
                                   bufs=2)
                     nc.sync.dma_start(
                         out=qTp,
                         in_=abuf[:, pi, 0].rearrange("s (r q) -> r s q", q=TC))
                     aT_all = pt.tile([64, 8, TC], BF16, tag="aT_all",
                                      name="aT_all", bufs=2)
                     for qc in range(8):
                         ps_a = pt_ps.tile([128, TC], F32, tag="psa",
                                           name="ps_a", bufs=2)
                         nb = 2 * (qc + 1)
                         for b in range(nb):
                             ps2 = pt_ps.tile([128, 1024], F32,
                                              tag=f"pe{b % 2}", name="ps2",
                                              bufs=1)
                             for i in range(2):
                                 kt = 2 * b + i
                                 nc.tensor.matmul(
                                     ps2[:, ts(i, TC)],
                                     kTp[:, kt // 4, ts(kt % 4, 128)],
                                     qTp[:, qc, :], start=True, stop=True)
                             pexp = pt.tile([128, 1024], BF16, tag="pexp2",
                                            name="pexp", bufs=3)
                             nc.scalar.activation(pexp, ps2, AF.Exp)
                             if b >= nb - 2:
                                 nc.vector.tensor_tensor(
                                     pexp, pexp, mkt_sb[:, b - (nb - 2), :],
                                     OP.mult)
                             for i in range(2):
                                 kt = 2 * b + i
                                 nc.tensor.matmul(ps_a[0:65, :],
                                                  vp[:, kt, :],
                                                  pexp[:, ts(i, TC)],
                                                  start=(kt == 0),
                                                  stop=(kt == 4 * qc + 3))
                         rec1 = pt.tile([1, TC], BF16, tag="rec1", name="rec1")
                         nc.vector.reciprocal(rec1, ps_a[64:65, :])
                         rb1 = pt.tile([64, TC], BF16, tag="rb1", name="rb1")
                         nc.gpsimd.partition_broadcast(rb1, rec1)
                         nc.vector.tensor_tensor(aT_all[:, qc, :],
                                                 ps_a[0:64, :], rb1, OP.mult)
                     nc.sync.dma_start(
                         out=aAiP[p].rearrange("s r q -> r s q"),
                         in_=aT_all)
                     nc.gpsimd.collective_compute(
                         "AllToAll", OP.bypass, replica_groups=RG,
                         ins=[aAiP[p].opt()], outs=[aAoP[p].opt()])
                 if dbg:
                     for p in range(8):
                         nc.sync.dma_start(
                             out=aAod[:, p], in_=aAoP[p])

             # ---- fused Wo_t + residual + MLP per channel ----
             if "BM" in PH:
              with (tc.tile_pool(name="pm", bufs=2) as pm,
                   tc.tile_pool(name="pm1", bufs=1) as pm1,
                   tc.tile_pool(name="pm_ps", bufs=1, space="PSUM") as pm_ps):
                 for c in range(C):
                     rhsAe = {}
                     for et in (0, 4, 1, 5, 2, 6, 3, 7):
                         re_t = pm.tile([128, TC], BF16, tag="rhsAe",
                                        name="re_t", bufs=8)
                         p0 = 2 * (et % 4)
                         nc.sync.dma_start(out=re_t[0:64, :],
                                           in_=aAoP[p0][2 * c + et // 4])
                         nc.sync.dma_start(out=re_t[64:128, :],
                                           in_=aAoP[p0 + 1][2 * c + et // 4])
                         rhsAe[et] = re_t
                     x1c = pm.tile([128, 8, TC], BF16, tag="x1c", name="x1c",
                                   bufs=2)
                     nc.sync.dma_start(
                         out=x1c,
                         in_=x1cm.rearrange("(k p) t -> p k t", p=128)[:, :, ts(c, TC)])
                     x2c = pm1.tile([128, 8, TC], BF16, tag="x2c", name="x2c",
                                    bufs=2)
                     for dt in range(8):
                         w_t = pm.tile([128, 8, 128], BF16, tag="wstream",
                                       name="w_t", bufs=3)
                         nc.sync.dma_start(out=w_t, in_=wot[dt])
                         ps_o = pm_ps.tile([128, TC], F32, tag="ps5",
                                           name="ps_o", bufs=2)
                         for i2, et in enumerate((0, 4, 1, 5, 2, 6, 3, 7)):
                             nc.tensor.matmul(ps_o, w_t[:, et, :],
                                              rhsAe[et],
                                              start=(i2 == 0), stop=(i2 == 7))
                         nc.vector.tensor_tensor(x2c[:, dt, :], ps_o,
                                                 x1c[:, dt, :], OP.add)
                     if dbg and c == 0:
                         pass
                         nc.sync.dma_start(out=x2Dbg, in_=x2c)
                     n_m = layernorm(pm, pm_ps, x2c, gbm_sb)
                     h_m = pm1.tile([128, 32, TC], BF16, tag="h_m", name="h_m",
                                    bufs=1)
                     for ft in range(32):
                         w1_t = pm.tile([128, 8, 128], BF16, tag="wstream",
                                        name="w1_t", bufs=3)
                         nc.sync.dma_start(out=w1_t, in_=w1t[ft])
                         ps1 = pm_ps.tile([128, TC], F32, tag="ps5",
                                          name="ps1", bufs=2)
                         for kt in range(8):
                             nc.tensor.matmul(ps1, w1_t[:, kt, :],
                                              n_m[:, kt, :],
                                              start=(kt == 0), stop=(kt == 7))
                         nc.vector.tensor_scalar(h_m[:, ft, :], ps1,
                                                 b1_sb[:, ft:ft + 1], 0.0,
                                                 OP.add, OP.max)
                     if dbg and c == 0:
                         nc.sync.dma_start(out=nmDbg, in_=n_m)
                         nc.sync.dma_start(out=hDbg, in_=h_m)
                     # one full-width accumulation group per PSUM bank
                     for dh in range(2):
                         psD = []
                         for i in range(4):
                             pd = pm_ps.tile([128, TC], F32, tag=f"psD{i}",
                                             name=f"psD{i}", bufs=1)
                             psD.append(pd)
                         for ft in range(32):
                             for i in range(4):
                                 dt = dh * 4 + i
                                 nc.tensor.matmul(
                                     psD[i], w2_sb[:, ft, ts(dt, 128)],
                                     h_m[:, ft, :],
                                     start=(ft == 0), stop=(ft == 31))
                         y_c = pm.tile([128, 4, TC], F32, tag="y_c",
                                       name="y_c", bufs=1)
                         for i in range(4):
                             dt = dh * 4 + i
                             nc.vector.scalar_tensor_tensor(
                                 y_c[:, i, :], psD[i],
                                 b2_sb[:, dt:dt + 1],
                                 x2c[:, dt, :], OP.add, OP.add)
                         nc.sync.dma_start(
                             out=yT.rearrange("(k p) t -> p k t", p=128)[:, dh * 4:dh * 4 + 4, ts(c, TC)],
                             in_=y_c)
        cst_cm.__exit__(None, None, None)

    nc.finalize()
    in_names = ["xT", "wqc", "wkc", "wvc", "woc", "wqt", "wkt", "wvt",
                "wot", "w1t", "w2r", "gb_c", "gb_t", "gb_m", "b1v", "b2v",
                "cq", "sq", "ck", "sk", "mkc4", "mkt2"]
    return nc, in_names


def _host_prep(inputs):
    """Build per-core in_maps from full inputs."""
    import ml_dtypes
    BF = ml_dtypes.bfloat16
    x = np.asarray(inputs["x"], np.float32)
    positions = np.asarray(inputs["positions"]).astype(np.int64)

    def T(a):
        return np.ascontiguousarray(np.asarray(a, np.float32).T)

    def tile8(wT):          # [1024, E] -> [128, 8, E]
        return np.ascontiguousarray(
            wT.reshape(8, 128, -1).transpose(1, 0, 2))

    def tile_et(wT):        # [1024, 1024] -> [8(et), 128, 8(kt), 128]
        return np.ascontiguousarray(
            tile8(wT).reshape(128, 8, 8, 128).transpose(2, 0, 1, 3))

    # temporal Q/K column permutation: [all evens (h-major, freq), all odds]
    perm = np.zeros(D, np.int64)
    for h in range(H_T):
        for i in range(32):
            perm[h * 32 + i] = h * 64 + 2 * i
            perm[512 + h * 32 + i] = h * 64 + 2 * i + 1
    wqtT = np.ascontiguousarray(T(inputs["Wq_t"])[:, perm])
    wktT = np.ascontiguousarray(T(inputs["Wk_t"])[:, perm])

    def gb(g, b):
        return np.ascontiguousarray(
            np.stack([np.asarray(g, np.float32), np.asarray(b, np.float32)],
                     axis=1))

    w1T = T(inputs["W1"])            # [1024, 4096]
    w1_tiled = np.ascontiguousarray(
        tile8(w1T).reshape(128, 8, 32, 128).transpose(2, 0, 1, 3))
    w2T = T(inputs["W2"])            # [4096, 1024]
    w2_res = np.ascontiguousarray(w2T.reshape(32, 128, D).transpose(1, 0, 2))

    shared = {
        "wqc": tile8(T(inputs["Wq_c"])).astype(BF),
        "wkc": tile8(T(inputs["Wk_c"])).astype(BF),
        "wvc": tile8(T(inputs["Wv_c"])).astype(BF),
        "woc": tile8(T(inputs["Wo_c"])).astype(BF),
        "wqt": tile_et(wqtT).astype(BF),
        "wkt": tile_et(wktT).astype(BF),
        "wvt": tile_et(T(inputs["Wv_t"])).astype(BF),
        "wot": tile_et(T(inputs["Wo_t"])).astype(BF),
        "w1t": w1_tiled.astype(BF),
        "w2r": w2_res.astype(BF),
        "gb_c": gb(inputs["g_c"], inputs["b_c"]),
        "gb_t": gb(inputs["g_t"], inputs["b_t"]),
        "gb_m": gb(inputs["g_m"], inputs["b_m"]),
        "b1v": np.asarray(inputs["b1"], np.float32).reshape(F_MLP, 1),
        "b2v": np.asarray(inputs["b2"], np.float32).reshape(D, 1),
    }
    # channel block-diag mask (tokens s-major, groups of 4), tiled 4 qt
    idx = np.arange(128)
    mkc = (idx[:, None] // 4 == idx[None, :] // 4).astype(np.float32)
    shared["mkc4"] = np.tile(mkc, (1, 4)).astype(BF)
    # temporal causal masks: batches of two 128-row k-tiles
    dq = np.arange(TC)
    dk = np.arange(128)
    mkt2 = np.zeros((2, 128, 1024), np.float32)
    for b in range(2):
        for i in range(2):
            r = 2 * b + i
            mkt2[b][:, i * TC:(i + 1) * TC] = (
                dq[None, :] >= r * 128 + dk[:, None]).astype(np.float32)
    shared["mkt2"] = mkt2.astype(BF)

    inv_freq = (10000.0 ** (-np.arange(32, dtype=np.float64) * 2 / HD_T))
    in_maps = []
    for i in range(N_CORES):
        m = dict(shared)
        xs = x[i * SB:(i + 1) * SB].reshape(TL, D)
        m["xT"] = np.ascontiguousarray(xs.T).astype(BF)
        pos = positions[i * SB:(i + 1) * SB].astype(np.float64)
        ang = pos[:, None] * inv_freq[None, :]          # [512, 32]
        cosT = np.cos(ang).T.astype(np.float32)         # [32, 512]
        sinT = np.sin(ang).T.astype(np.float32)
        c4 = np.tile(cosT, (4, 1))                      # [128, 512]
        s4 = np.tile(sinT, (4, 1))
        m["cq"] = np.tile((c4 * 0.125)[:, None, :], (1, 4, 1)).astype(BF)
        m["sq"] = np.tile((s4 * 0.125)[:, None, :], (1, 4, 1)).astype(BF)
        m["ck"] = np.tile(c4[:, None, :], (1, 4, 1)).astype(BF)
        m["sk"] = np.tile(s4[:, None, :], (1, 4, 1)).astype(BF)
        in_maps.append(m)
    return in_maps


def _run(inputs, trace=False):
    from concourse.bass_utils import run_bass_kernel_spmd
    if "prog" not in _CACHE:
        _CACHE["prog"] = _build_program()
    nc, in_names = _CACHE["prog"]
    in_maps = _host_prep(inputs)
    for m in in_maps:
        for k in list(m.keys()):
            assert k in in_names, k
    res = run_bass_kernel_spmd(nc, in_maps, core_ids=list(range(N_CORES)),
                               trace=trace)
    out = np.zeros((S, C, D), np.float32)
    for i in range(N_CORES):
        yT = res.results[i]["yT"]                        # [1024, 2048] c-major
        yi = yT.T.reshape(C, SB, D)                      # [c, s, d]
        out[i * SB:(i + 1) * SB] = yi.transpose(1, 0, 2)
    return out, res


def kernel(**inputs) -> np.ndarray:
    out, _ = _run(inputs, trace=False)
    return out


# revision 47
# speedup vs baseline: 1.9567x; 1.0071x over previous
"""AxialTransformerBlock Trainium2 kernel (8 NeuronCores, SPMD + AllToAll).

Sharding: sequence-parallel over S (512 rows/core) for LN / channel attention /
MLP; head-parallel via one fused bf16 AllToAll for temporal causal attention
(8 (c,h) pairs per core over the full sequence), bf16 AllToAll back, then a
fused Wo_t + MLP pass per channel that keeps the residual in SBUF.

On-device layout: feature-major residual stream x^T [D, T], bf16 activations
with fp32 PSUM accumulation. Host pre-transposes/pre-tiles weights to bf16,
bakes RoPE cos/sin tables (even/odd de-interleave folded into the Wq_t/Wk_t
column permutation) and causal / channel block-diagonal masks.
"""

import contextlib

import numpy as np

N_CORES = 8
S, C, D = 4096, 4, 1024
SB = S // N_CORES          # 512 s-rows per core
TL = SB * C                # 2048 local tokens
H_T, HD_T = 16, 64
H_C, HD_C = 4, 256
F_MLP = 4 * D              # 4096
LN_EPS = 1e-5
TC = 512                   # token chunk
NCH = TL // TC             # 4
MC = 256                   # MLP second-gemm sub-chunk

_CACHE = {}


def _build_program():
    import concourse.bass as bass
    import concourse.bacc as bacc
    import concourse.tile as tile
    from concourse import mybir

    F32 = mybir.dt.float32
    BF16 = mybir.dt.bfloat16
    AF = mybir.ActivationFunctionType
    OP = mybir.AluOpType
    ts = bass.ts

    nc = bacc.Bacc("TRN2", target_bir_lowering=False, debug=False,
                   num_devices=N_CORES)

    def din(name, shape, dt=BF16):
        return nc.dram_tensor(name, list(shape), dt, kind="ExternalInput").ap()

    xT = din("xT", [D, TL])
    # phase-A weights, resident layout [128, kt, e_out]
    wqc = din("wqc", [128, 8, D])
    wkc = din("wkc", [128, 8, D])
    wvc = din("wvc", [128, 8, D])
    woc = din("woc", [128, 8, D])
    # phase-B projection weights, streamed layout [et, 128, kt, 128]
    wqt = din("wqt", [8, 128, 8, 128])
    wkt = din("wkt", [8, 128, 8, 128])
    wvt = din("wvt", [8, 128, 8, 128])
    # Wo_t streamed per output tile dt
    wot = din("wot", [8, 128, 8, 128])
    # MLP: W1 streamed per ft, W2 resident
    w1t = din("w1t", [32, 128, 8, 128])
    w2r = din("w2r", [128, 32, D])
    gb_c = din("gb_c", [D, 2], F32)   # col0 = g, col1 = b
    gb_t = din("gb_t", [D, 2], F32)
    gb_m = din("gb_m", [D, 2], F32)
    b1v = din("b1v", [F_MLP, 1], F32)
    b2v = din("b2v", [D, 1], F32)
    cq_d = din("cq", [128, 4, TC])
    sq_d = din("sq", [128, 4, TC])
    ck_d = din("ck", [128, 4, TC])
    sk_d = din("sk", [128, 4, TC])
    mkc_d = din("mkc4", [128, TC])
    mkt_d = din("mkt2", [2, 128, 1024])

    yT = nc.dram_tensor("yT", [D, TL], F32, kind="ExternalOutput").ap()

    import os
    dbg = os.environ.get("KDBG", "0") == "1"
    kindd = "ExternalOutput" if dbg else "Internal"
    PH = set(os.environ.get("KPHASES", "A,Bf,Ba,BM").split(","))
    x1cm = nc.dram_tensor("x1cm", [D, TL], BF16, kind=kindd).ap()
    # fused QKV all-to-all payload: per (dest, slot): sec0=Q[64,512],
    # sec1=K[64,512], sec2=V[512,64] (flat bytes)
    a2aIh = [nc.dram_tensor(f"a2aI{i}", [8, 4, 3, 64 * TC], BF16).ap()
             for i in range(2)]
    a2aOh = [nc.dram_tensor(f"a2aO{i}", [8, 4, 3, 64 * TC], BF16).ap()
             for i in range(2)]
    aAiP = [nc.dram_tensor(f"aAi{i}", [8, 64, TC], BF16).ap()
            for i in range(8)]
    aAoP = [nc.dram_tensor(f"aAo{i}", [8, 64, TC], BF16).ap()
            for i in range(8)]
    if dbg:
        a2aOd = nc.dram_tensor("a2aOd", [8, 8, 3, 64 * TC], BF16,
                               kind="ExternalOutput").ap()
        aAod = nc.dram_tensor("aAod", [8, 8, 64, TC], BF16,
                              kind="ExternalOutput").ap()
        xDbg = nc.dram_tensor("xDbg", [128, 8, TC], BF16,
                              kind="ExternalOutput").ap()
        nDbg = nc.dram_tensor("nDbg", [128, 8, TC], BF16,
                              kind="ExternalOutput").ap()
        qDbg = nc.dram_tensor("qDbg", [128, 8, TC], BF16,
                              kind="ExternalOutput").ap()
        kDbg = nc.dram_tensor("kDbg", [128, 8, TC], BF16,
                              kind="ExternalOutput").ap()
        vDbg = nc.dram_tensor("vDbg", [128, 4, D], BF16,
                              kind="ExternalOutput").ap()
        aDbg = nc.dram_tensor("aDbg", [128, 8, TC], BF16,
                              kind="ExternalOutput").ap()
        pDbg = nc.dram_tensor("pDbg", [128, TC], BF16,
                              kind="ExternalOutput").ap()
        rbDbg = nc.dram_tensor("rbDbg", [128, TC], BF16,
                               kind="ExternalOutput").ap()
        x2Dbg = nc.dram_tensor("x2Dbg", [128, 8, TC], BF16,
                               kind="ExternalOutput").ap()
        nmDbg = nc.dram_tensor("nmDbg", [128, 8, TC], BF16,
                               kind="ExternalOutput").ap()
        hDbg = nc.dram_tensor("hDbg", [128, 32, TC], BF16,
                              kind="ExternalOutput").ap()
        raDbg = nc.dram_tensor("raDbg", [128, 8, TC], BF16,
                               kind="ExternalOutput").ap()

    RG = [list(range(N_CORES))]

    with tile.TileContext(nc) as tc, \
            nc.allow_low_precision(reason="bf16 kernel; 2e-2 tolerance"):
        cst_cm = tc.tile_pool(name="cst", bufs=1)
        cst = cst_cm.__enter__()
        ones_mean = cst.tile([128, 1], BF16)      # 1/1024: stats matmuls
        nc.vector.memset(ones_mean, 1.0 / D)
        ones_one = cst.tile([128, 1], BF16)       # 1.0: channel-attn denom
        nc.vector.memset(ones_one, 1.0)
        eps1 = cst.tile([1, 1], F32)
        nc.vector.memset(eps1, LN_EPS)
        gbc_sb = cst.tile([128, 8, 2], F32)
        nc.sync.dma_start(out=gbc_sb, in_=gb_c.rearrange("(k p) two -> p k two", p=128))
        gbt_sb = cst.tile([128, 8, 2], F32)
        nc.sync.dma_start(out=gbt_sb, in_=gb_t.rearrange("(k p) two -> p k two", p=128))
        gbm_sb = cst.tile([128, 8, 2], F32)
        nc.sync.dma_start(out=gbm_sb, in_=gb_m.rearrange("(k p) two -> p k two", p=128))
        b1_sb = cst.tile([128, 32], F32)
        nc.sync.dma_start(out=b1_sb, in_=b1v.rearrange("(k p) one -> p (k one)", p=128))
        b2_sb = cst.tile([128, 8], F32)
        nc.sync.dma_start(out=b2_sb, in_=b2v.rearrange("(k p) one -> p (k one)", p=128))
        mkc_sb = cst.tile([128, TC], BF16)
        nc.sync.dma_start(out=mkc_sb, in_=mkc_d)
        mkt_sb = cst.tile([128, 2, 1024], BF16)
        nc.sync.dma_start(out=mkt_sb, in_=mkt_d.rearrange("b p q -> p b q"))

        def layernorm(pool, psum, x_ch, gb_sb):
            """x_ch [128, 8, TC] bf16 -> n_ch bf16 same shape."""
            st_s = psum.tile([128, TC], F32, tag="ps5", name="st_s", bufs=2)
            st_q = psum.tile([128, TC], F32, tag="ps5", name="st_q", bufs=2)
            for kt in range(8):
                nc.tensor.matmul(st_s[0:1, :], ones_mean, x_ch[:, kt, :],
                                 start=(kt == 0), stop=(kt == 7))
            for kt in range(8):
                xsq = pool.tile([128, TC], BF16, tag="ln_xsq", name="xsq",
                                bufs=2)
                nc.vector.tensor_tensor(xsq, x_ch[:, kt, :], x_ch[:, kt, :],
                                        OP.mult)
                nc.tensor.matmul(st_q[0:1, :], ones_mean, xsq,
                                 start=(kt == 0), stop=(kt == 7))
            mu2 = pool.tile([1, TC], F32, tag="ln_mu2", name="mu2")
            nc.scalar.activation(mu2, st_s[0:1, :], AF.Square)
            var = pool.tile([1, TC], F32, tag="ln_var", name="var")
            nc.vector.tensor_tensor(var, st_q[0:1, :], mu2, OP.subtract)
            sd = pool.tile([1, TC], F32, tag="ln_sd", name="sd")
            nc.scalar.activation(sd, var, AF.Sqrt, bias=eps1)
            rs = pool.tile([1, TC], BF16, tag="ln_rs", name="rs")
            nc.vector.reciprocal(rs, sd)
            bv = pool.tile([1, TC], BF16, tag="ln_bv", name="bv")
            nc.vector.tensor_tensor(bv, st_s[0:1, :], rs, OP.mult)
            ab = pool.tile([128, TC], BF16, tag="ln_ab", name="ab")
            nc.gpsimd.partition_broadcast(ab, rs)
            bb = pool.tile([128, TC], BF16, tag="ln_bb", name="bb")
            nc.gpsimd.partition_broadcast(bb, bv)
            n_ch = pool.tile([128, 8, TC], BF16, tag="ln_out", name="n_ch",
                             bufs=2)
            for kt in range(8):
                t1 = pool.tile([128, TC], BF16, tag="ln_t1", name="t1", bufs=2)
                nc.vector.tensor_tensor(t1, x_ch[:, kt, :], ab, OP.mult)
                nc.vector.tensor_tensor(t1, t1, bb, OP.subtract)
                nc.vector.tensor_scalar(n_ch[:, kt, :], t1,
                                        gb_sb[:, kt, 0:1], gb_sb[:, kt, 1:2],
                                        OP.mult, OP.add)
            return n_ch

        def proj_fmajor_res(psum, w_sb, n_ch, out_ch):
            """Resident weights [128, 8, D]: out_ch[:, et, :] feature-major."""
            for et in range(8):
                ps = psum.tile([128, TC], F32, tag="ps5", name="ps", bufs=2)
                for kt in range(8):
                    nc.tensor.matmul(ps, w_sb[:, kt, ts(et, 128)],
                                     n_ch[:, kt, :],
                                     start=(kt == 0), stop=(kt == 7))
                nc.scalar.activation(out_ch[:, et, :], ps, AF.Copy)

        def proj_fmajor_stream(pool, psum, wdram, n_ch, out_ch):
            """Streamed weights [et, 128, 8, 128]."""
            for et in range(8):
                w_t = pool.tile([128, 8, 128], BF16, tag="wstream", name="w_t",
                                bufs=4)
                nc.sync.dma_start(out=w_t, in_=wdram[et])
                ps = psum.tile([128, TC], F32, tag="ps5", name="ps", bufs=2)
                for kt in range(8):
                    nc.tensor.matmul(ps, w_t[:, kt, :], n_ch[:, kt, :],
                                     start=(kt == 0), stop=(kt == 7))
                nc.scalar.activation(out_ch[:, et, :], ps, AF.Copy)

        def proj_tmajor(pool, psum, wsrc, n_ch, out_ch, resident):
            """V token-major: out_ch [128, 4, 1024]."""
            for ec in range(2):
                psv = []
                for tt in range(4):
                    pv = psum.tile([128, TC], F32, tag="psv", name=f"pv{tt}",
                                   bufs=4)
                    psv.append(pv)
                for kt in range(8):
                    if resident:
                        w_mv = wsrc[:, kt, ts(ec, TC)]
                    else:
                        w_t = pool.tile([128, TC], BF16, tag="wstreamV",
                                        name="w_tv", bufs=4)
                        nc.sync.dma_start(
                            out=w_t.rearrange("p (a e) -> p a e", e=128),
                            in_=wsrc[4 * ec:4 * ec + 4, :, kt, :].rearrange(
                                "et p e -> p et e"))
                        w_mv = w_t
                    for tt in range(4):
                        nc.tensor.matmul(psv[tt], n_ch[:, kt, ts(tt, 128)],
                                         w_mv, start=(kt == 0), stop=(kt == 7))
                for tt in range(4):
                    nc.scalar.activation(out_ch[:, tt, ts(ec, TC)], psv[tt],
                                         AF.Copy)

        # ---------------- Phase A: channel attention ----------------
        if "A" in PH:
         with (tc.tile_pool(name="wa", bufs=1) as wa,
              tc.tile_pool(name="pa", bufs=2) as pa,
              tc.tile_pool(name="pa1", bufs=1) as pa1,
              tc.tile_pool(name="pa_ps", bufs=1, space="PSUM") as pa_ps):
             x_pre = []
             for ch in range(2):
                 xt = pa.tile([128, 8, TC], BF16, tag="x_ch", name="x_ch",
                              bufs=2)
                 nc.sync.dma_start(
                     out=xt,
                     in_=xT.rearrange("(k p) t -> p k t", p=128)[:, :, ts(ch, TC)])
                 x_pre.append(xt)
             wqc_sb = wa.tile([128, 8, D], BF16, tag="wqc", name="wqc_sb")
             nc.sync.dma_start(out=wqc_sb, in_=wqc)
             wkc_sb = wa.tile([128, 8, D], BF16, tag="wkc", name="wkc_sb")
             nc.sync.dma_start(out=wkc_sb, in_=wkc)
             wvc_sb = wa.tile([128, 8, D], BF16, tag="wvc", name="wvc_sb")
             nc.sync.dma_start(out=wvc_sb, in_=wvc)
             woc_sb = wa.tile([128, 8, D], BF16, tag="woc", name="woc_sb")
             nc.sync.dma_start(out=woc_sb, in_=woc)
             for ch in range(NCH):
                 if ch < 2:
                     x_ch = x_pre[ch]
                 else:
                     x_ch = pa.tile([128, 8, TC], BF16, tag="x_ch",
                                    name="x_ch", bufs=2)
                     nc.sync.dma_start(
                         out=x_ch,
                         in_=xT.rearrange("(k p) t -> p k t", p=128)[:, :, ts(ch, TC)])
                 n_ch = layernorm(pa, pa_ps, x_ch, gbc_sb)
                 q_ch = pa1.tile([128, 8, TC], BF16, tag="q_ch", name="q_ch")
                 proj_fmajor_res(pa_ps, wqc_sb, n_ch, q_ch)
                 k_ch = pa1.tile([128, 8, TC], BF16, tag="k_ch", name="k_ch")
                 proj_fmajor_res(pa_ps, wkc_sb, n_ch, k_ch)
                 v_ch = pa1.tile([128, 4, D], BF16, tag="v_ch", name="v_ch")
                 proj_tmajor(pa, pa_ps, wvc_sb, n_ch, v_ch, resident=True)
                 if dbg and ch == 0:
                     nc.sync.dma_start(out=xDbg, in_=x_ch)
                     nc.sync.dma_start(out=nDbg, in_=n_ch)
                     nc.sync.dma_start(out=qDbg, in_=q_ch)
                     nc.sync.dma_start(out=kDbg, in_=k_ch)
                     nc.sync.dma_start(out=vDbg, in_=v_ch)

                 # attention: logits computed k-major [k, q], block-diag mask
                 aT_ch = pa1.tile([128, 8, TC], BF16, tag="aT_ch",
                                  name="aT_ch")
                 for h in range(H_C):
                     ps_l = pa_ps.tile([128, TC], F32, tag="psx", name="ps_l",
                                       bufs=2)
                     for qt in range(4):
                         for i, et in enumerate((2 * h, 2 * h + 1)):
                             nc.tensor.matmul(ps_l[:, ts(qt, 128)],
                                              k_ch[:, et, ts(qt, 128)],
                                              q_ch[:, et, ts(qt, 128)],
                                              start=(i == 0), stop=(i == 1))
                     pexp = pa.tile([128, TC], BF16, tag="pexp", name="pexp")
                     nc.scalar.activation(pexp, ps_l, AF.Exp, scale=1.0 / 16.0)
                     nc.vector.tensor_tensor(pexp, pexp, mkc_sb, OP.mult)
                     den = pa_ps.tile([128, TC], F32, tag="ps5", name="den",
                                      bufs=2)
                     nc.tensor.matmul(den[0:1, :], ones_one, pexp,
                                      start=True, stop=True)
                     rec = pa.tile([1, TC], BF16, tag="rec", name="rec")
                     nc.vector.reciprocal(rec, den[0:1, :])
                     rb = pa.tile([128, TC], BF16, tag="rb", name="rb")
                     nc.gpsimd.partition_broadcast(rb, rec)
                     if dbg and ch == 0 and h == 0:
                         nc.sync.dma_start(out=pDbg, in_=pexp)
                         nc.sync.dma_start(out=rbDbg, in_=rb)
                     for i, es in enumerate((2 * h, 2 * h + 1)):
                         ps_av = pa_ps.tile([128, TC], F32, tag="psv",
                                            name="ps_av", bufs=4)
                         for qt in range(4):
                             nc.tensor.matmul(ps_av[:, ts(qt, 128)],
                                              v_ch[:, qt, ts(es, 128)],
                                              pexp[:, ts(qt, 128)],
                                              start=True, stop=True)
                         # evict + normalize in one DVE op
                         nc.vector.tensor_tensor(aT_ch[:, es, :], ps_av, rb,
                                                 OP.mult)
                 # Wo + residual, write c-major bf16
                 if dbg and ch == 0:
                     nc.sync.dma_start(out=aDbg, in_=aT_ch)
                 x1w = pa.tile([128, 8, 4, 128], BF16, tag="x1w", name="x1w")
                 for dt in range(8):
                     ps_o = pa_ps.tile([128, TC], F32, tag="ps5", name="ps_o",
                                       bufs=2)
                     for et in range(8):
                         nc.tensor.matmul(ps_o, woc_sb[:, et, ts(dt, 128)],
                                          aT_ch[:, et, :],
                                          start=(et == 0), stop=(et == 7))
                     nc.vector.tensor_tensor(
                         x1w[:, dt].rearrange("p c s -> p s c"),
                         ps_o.rearrange("p (s c) -> p s c", c=4),
                         x_ch[:, dt, :].rearrange("p (s c) -> p s c", c=4),
                         OP.add)
                 for dt in range(8):
                     nc.sync.dma_start(
                         out=x1cm.rearrange("(k p) (c u) -> p k c u", p=128,
                                            c=4)[:, dt, :, ts(ch, 128)],
                         in_=x1w[:, dt])

        # ---------------- Phase B: temporal attention ----------------
        if PH & {"Bf", "Ba", "BM"}:
         with contextlib.ExitStack() as _bstk:
             if "Bf" in PH:
              with (tc.tile_pool(name="pb", bufs=2) as pb,
                   tc.tile_pool(name="pb1", bufs=1) as pb1,
                   tc.tile_pool(name="pb_ps", bufs=1, space="PSUM") as pb_ps):
                 cq_sb = pb1.tile([128, 4, TC], BF16, tag="cq", name="cq_sb")
                 nc.sync.dma_start(out=cq_sb, in_=cq_d)
                 sq_sb = pb1.tile([128, 4, TC], BF16, tag="sq", name="sq_sb")
                 nc.sync.dma_start(out=sq_sb, in_=sq_d)
                 ck_sb = pb1.tile([128, 4, TC], BF16, tag="ck", name="ck_sb")
                 nc.sync.dma_start(out=ck_sb, in_=ck_d)
                 sk_sb = pb1.tile([128, 4, TC], BF16, tag="sk", name="sk_sb")
                 nc.sync.dma_start(out=sk_sb, in_=sk_d)
                 x1_pre = pb.tile([128, 8, TC], BF16, tag="x_ch",
                                  name="x1_ch", bufs=2)
                 nc.sync.dma_start(
                     out=x1_pre,
                     in_=x1cm.rearrange("(k p) t -> p k t", p=128)[:, :, ts(0, TC)])
                 for c in range(C):
                     x1_ch = x1_pre
                     if c + 1 < C:
                         x1_pre = pb.tile([128, 8, TC], BF16, tag="x_ch",
                                          name="x1_ch", bufs=2)
                         nc.sync.dma_start(
                             out=x1_pre,
                             in_=x1cm.rearrange("(k p) t -> p k t", p=128)[:, :, ts(c + 1, TC)])
                     n_ch = layernorm(pb, pb_ps, x1_ch, gbt_sb)
                     q_ch = pb1.tile([128, 8, TC], BF16, tag="q_ch",
                                     name="q_ch", bufs=2)
                     proj_fmajor_stream(pb, pb_ps, wqt, n_ch, q_ch)
                     k_ch = pb1.tile([128, 8, TC], BF16, tag="k_ch",
                                     name="k_ch", bufs=2)
                     proj_fmajor_stream(pb, pb_ps, wkt, n_ch, k_ch)
                     v_ch = pb1.tile([128, 4, D], BF16, tag="v_ch",
                                     name="v_ch", bufs=2)
                     proj_tmajor(pb, pb_ps, wvt, n_ch, v_ch, resident=False)
                     # RoPE in place (ev tiles 0..3, od tiles 4..7)
                     for tgt, cos_sb, sin_sb in ((q_ch, cq_sb, sq_sb),
                                                 (k_ch, ck_sb, sk_sb)):
                         ev = tgt[:, 0:4, :]
                         od = tgt[:, 4:8, :]
                         t1 = pb.tile([128, 4, TC], BF16, tag="rp1", name="t1")
                         t2 = pb.tile([128, 4, TC], BF16, tag="rp2", name="t2")
                         t3 = pb.tile([128, 4, TC], BF16, tag="rp3", name="t3")
                         t4 = pb.tile([128, 4, TC], BF16, tag="rp4", name="t4")
                         nc.vector.tensor_tensor(t1, ev, cos_sb, OP.mult)
                         nc.vector.tensor_tensor(t2, ev, sin_sb, OP.mult)
                         nc.vector.tensor_tensor(t3, od, sin_sb, OP.mult)
                         nc.vector.tensor_tensor(t4, od, cos_sb, OP.mult)
                         nc.vector.tensor_tensor(ev, t1, t3, OP.subtract)
                         nc.vector.tensor_tensor(od, t2, t4, OP.add)
                     # scatter to the fused A2A buffer (6 DMAs per channel)
                     for j2 in range(2):
                         j = 2 * c + j2
                         for sec, src in ((0, q_ch), (1, k_ch)):
                             for f in range(2):
                                 for kt in range(2):
                                     nc.sync.dma_start(
                                         out=a2aIh[kt][j, :, sec].rearrange(
                                             "pr (f r q) -> f pr r q",
                                             f=2, q=TC)[f],
                                         in_=src[:, f * 4 + 2 * j2 + kt, :])
                         for tt in range(4):
                             for hf in range(2):
                                 nc.scalar.dma_start(
                                     out=a2aIh[hf][j, :, 2].rearrange(
                                         "pl (tt p hd) -> tt p pl hd",
                                         p=128, hd=64)[tt],
                                     in_=v_ch[:, tt, j2 * TC + hf * 256:
                                              j2 * TC + hf * 256 + 256].rearrange(
                                         "p (pl hd) -> p pl hd", hd=64))
                 for hf in range(2):
                     nc.gpsimd.collective_compute(
                         "AllToAll", OP.bypass, replica_groups=RG,
                         ins=[a2aIh[hf].opt()], outs=[a2aOh[hf].opt()])
                 if dbg:
                     for hf in range(2):
                         nc.sync.dma_start(
                             out=a2aOd.rearrange(
                                 "s (hf pl) sec e -> hf s pl sec e", hf=2)[hf],
                             in_=a2aOh[hf])
             # W2 resident pool opens after Bf pools close; its DMA has no
             # dependency on the collective so it overlaps it
             wm = _bstk.enter_context(tc.tile_pool(name="wm", bufs=1))
             w2_sb = wm.tile([128, 32, D], BF16, tag="w2r", name="w2_sb")
             nc.sync.dma_start(out=w2_sb, in_=w2r)

             # flash attention per local pair over full S
             if "Ba" in PH:
              with (tc.tile_pool(name="pt", bufs=2) as pt,
                   tc.tile_pool(name="pt_ps", bufs=1, space="PSUM") as pt_ps):
                 for p in range(8):
                     abuf = a2aOh[p // 4]
                     pi = p % 4
                     kTp = pt.tile([64, 8, TC], BF16, tag="kTp", name="kTp",
                                   bufs=2)
                     nc.sync.dma_start(
                         out=kTp,
                         in_=abuf[:, pi, 1].rearrange("s (r q) -> r s q", q=TC))
                     vp = pt.tile([128, 32, 65], BF16, tag="vp", name="vp",
                                  bufs=2)
                     for k4 in range(4):
                         nc.sync.dma_start(
                             out=vp[:, :, 0:64].rearrange(
                                 "p (s k4) hd -> p s k4 hd", k4=4)[:, :, k4],
                             in_=abuf[:, pi, 2].rearrange(
                                 "s (k4 p hd) -> k4 p s hd", p=128, hd=64)[k4])
                     nc.vector.memset(vp[:, :, 64:65], 1.0)
                     qTp = pt.tile([64, 8, TC], BF16, tag="qTp", name="qTp",
                                   bufs=2)
                     nc.sync.dma_start(
                         out=qTp,
                         in_=abuf[:, pi, 0].rearrange("s (r q) -> r s q", q=TC))
                     aT_all = pt.tile([64, 8, TC], BF16, tag="aT_all",
                                      name="aT_all", bufs=2)
                     for qc in range(8):
                         ps_a = pt_ps.tile([128, TC], F32, tag="psa",
                                           name="ps_a", bufs=2)
                         nb = 2 * (qc + 1)
                         for b in range(nb):
                             ps2 = pt_ps.tile([128, 1024], F32,
                                              tag=f"pe{b % 2}", name="ps2",
                                              bufs=1)
                             for i in range(2):
                                 kt = 2 * b + i
                                 nc.tensor.matmul(
                                     ps2[:, ts(i, TC)],
                                     kTp[:, kt // 4, ts(kt % 4, 128)],
                                     qTp[:, qc, :], start=True, stop=True)
                             pexp = pt.tile([128, 1024], BF16, tag="pexp2",
                                            name="pexp", bufs=3)
                             nc.scalar.activation(pexp, ps2, AF.Exp)
                             if b >= nb - 2:
                                 nc.vector.tensor_tensor(
                                     pexp, pexp, mkt_sb[:, b - (nb - 2), :],
                                     OP.mult)
                             for i in range(2):
                                 kt = 2 * b + i
                                 nc.tensor.matmul(ps_a[0:65, :],
                                                  vp[:, kt, :],
                                                  pexp[:, ts(i, TC)],
                                                  start=(kt == 0),
                                                  stop=(kt == 4 * qc + 3))
                         rec1 = pt.tile([1, TC], BF16, tag="rec1", name="rec1")
                         nc.vector.reciprocal(rec1, ps_a[64:65, :])
                         rb1 = pt.tile([64, TC], BF16, tag="rb1", name="rb1")
                         nc.gpsimd.partition_broadcast(rb1, rec1)
                         nc.vector.tensor_tensor(aT_all[:, qc, :],
                                                 ps_a[0:64, :], rb1, OP.mult)
                     nc.sync.dma_start(
                         out=aAiP[p].rearrange("s r q -> r s q"),
                         in_=aT_all)
                     nc.gpsimd.collective_compute(
                         "AllToAll", OP.bypass, replica_groups=RG,
                         ins=[aAiP[p].opt()], outs=[aAoP[p].opt()])
                 if dbg:
                     for p in range(8):
                         nc.sync.dma_start(
                             out=aAod[:, p], in_=aAoP[p])

             # ---- fused Wo_t + residual + MLP per channel ----
             if "BM" in PH:
              with (tc.tile_pool(name="pm", bufs=2) as pm,
                   tc.tile_pool(name="pm1", bufs=1) as pm1,
                   tc.tile_pool(name="pm_ps", bufs=1, space="PSUM") as pm_ps):
                 for c in range(C):
                     rhsAe = {}
                     for et in (0, 4, 1, 5, 2, 6, 3, 7):
                         re_t = pm.tile([128, TC], BF16, tag="rhsAe",
                                        name="re_t", bufs=8)
                         p0 = 2 * (et % 4)
                         nc.sync.dma_start(out=re_t[0:64, :],
                                           in_=aAoP[p0][2 * c + et // 4])
                         nc.sync.dma_start(out=re_t[64:128, :],
                                           in_=aAoP[p0 + 1][2 * c + et // 4])
                         rhsAe[et] = re_t
                     x1c = pm.tile([128, 8, TC], BF16, tag="x1c", name="x1c",
                                   bufs=2)
                     nc.sync.dma_start(
                         out=x1c,
                         in_=x1cm.rearrange("(k p) t -> p k t", p=128)[:, :, ts(c, TC)])
                     x2c = pm1.tile([128, 8, TC], BF16, tag="x2c", name="x2c",
                                    bufs=2)
                     for dt in range(8):
                         w_t = pm.tile([128, 8, 128], BF16, tag="wstream",
                                       name="w_t", bufs=4)
                         nc.sync.dma_start(out=w_t, in_=wot[dt])
                         ps_o = pm_ps.tile([128, TC], F32, tag="ps5",
                                           name="ps_o", bufs=2)
                         for i2, et in enumerate((0, 4, 1, 5, 2, 6, 3, 7)):
                             nc.tensor.matmul(ps_o, w_t[:, et, :],
                                              rhsAe[et],
                                              start=(i2 == 0), stop=(i2 == 7))
                         nc.vector.tensor_tensor(x2c[:, dt, :], ps_o,
                                                 x1c[:, dt, :], OP.add)
                     if dbg and c == 0:
                         pass
                         nc.sync.dma_start(out=x2Dbg, in_=x2c)
                     n_m = layernorm(pm, pm_ps, x2c, gbm_sb)
                     h_m = pm1.tile([128, 32, TC], BF16, tag="h_m", name="h_m",
                                    bufs=1)
                     for ft in range(32):
                         w1_t = pm.tile([128, 8, 128], BF16, tag="wstream",
                                        name="w1_t", bufs=4)
                         nc.sync.dma_start(out=w1_t, in_=w1t[ft])
                         ps1 = pm_ps.tile([128, TC], F32, tag="ps5",
                                          name="ps1", bufs=2)
                         for kt in range(8):
                             nc.tensor.matmul(ps1, w1_t[:, kt, :],
                                              n_m[:, kt, :],
                                              start=(kt == 0), stop=(kt == 7))
                         nc.vector.tensor_scalar(h_m[:, ft, :], ps1,
                                                 b1_sb[:, ft:ft + 1], 0.0,
                                                 OP.add, OP.max)
                     if dbg and c == 0:
                         nc.sync.dma_start(out=nmDbg, in_=n_m)
                         nc.sync.dma_start(out=hDbg, in_=h_m)
                     # one full-width accumulation group per PSUM bank
                     for dh in range(2):
                         psD = []
                         for i in range(4):
                             pd = pm_ps.tile([128, TC], F32, tag=f"psD{i}",
                                             name=f"psD{i}", bufs=1)
                             psD.append(pd)
                         for ft in range(32):
                             for i in range(4):
                                 dt = dh * 4 + i
                                 nc.tensor.matmul(
                                     psD[i], w2_sb[:, ft, ts(dt, 128)],
                                     h_m[:, ft, :],
                                     start=(ft == 0), stop=(ft == 31))
                         y_c = pm.tile([128, 4, TC], F32, tag="y_c",
                                       name="y_c", bufs=1)
                         for i in range(4):
                             dt = dh * 4 + i
                             nc.vector.scalar_tensor_tensor(
                                 y_c[:, i, :], psD[i],
                                 b2_sb[:, dt:dt + 1],
                                 x2c[:, dt, :], OP.add, OP.add)
                         nc.sync.dma_start(
                             out=yT.rearrange("(k p) t -> p k t", p=128)[:, dh * 4:dh * 4 + 4, ts(c, TC)],
                             in_=y_c)
        cst_cm.__exit__(None, None, None)

    nc.finalize()
    in_names = ["xT", "wqc", "wkc", "wvc", "woc", "wqt", "wkt", "wvt",
                "wot", "w1t", "w2r", "gb_c", "gb_t", "gb_m", "b1v", "b2v",
                "cq", "sq", "ck", "sk", "mkc4", "mkt2"]
    return nc, in_names


def _host_prep(inputs):
    """Build per-core in_maps from full inputs."""
    import ml_dtypes
    BF = ml_dtypes.bfloat16
    x = np.asarray(inputs["x"], np.float32)
    positions = np.asarray(inputs["positions"]).astype(np.int64)

    def T(a):
        return np.ascontiguousarray(np.asarray(a, np.float32).T)

    def tile8(wT):          # [1024, E] -> [128, 8, E]
        return np.ascontiguousarray(
            wT.reshape(8, 128, -1).transpose(1, 0, 2))

    def tile_et(wT):        # [1024, 1024] -> [8(et), 128, 8(kt), 128]
        return np.ascontiguousarray(
            tile8(wT).reshape(128, 8, 8, 128).transpose(2, 0, 1, 3))

    # temporal Q/K column permutation: [all evens (h-major, freq), all odds]
    perm = np.zeros(D, np.int64)
    for h in range(H_T):
        for i in range(32):
            perm[h * 32 + i] = h * 64 + 2 * i
            perm[512 + h * 32 + i] = h * 64 + 2 * i + 1
    wqtT = np.ascontiguousarray(T(inputs["Wq_t"])[:, perm])
    wktT = np.ascontiguousarray(T(inputs["Wk_t"])[:, perm])

    def gb(g, b):
        return np.ascontiguousarray(
            np.stack([np.asarray(g, np.float32), np.asarray(b, np.float32)],
                     axis=1))

    w1T = T(inputs["W1"])            # [1024, 4096]
    w1_tiled = np.ascontiguousarray(
        tile8(w1T).reshape(128, 8, 32, 128).transpose(2, 0, 1, 3))
    w2T = T(inputs["W2"])            # [4096, 1024]
    w2_res = np.ascontiguousarray(w2T.reshape(32, 128, D).transpose(1, 0, 2))

    shared = {
        "wqc": tile8(T(inputs["Wq_c"])).astype(BF),
        "wkc": tile8(T(inputs["Wk_c"])).astype(BF),
        "wvc": tile8(T(inputs["Wv_c"])).astype(BF),
        "woc": tile8(T(inputs["Wo_c"])).astype(BF),
        "wqt": tile_et(wqtT).astype(BF),
        "wkt": tile_et(wktT).astype(BF),
        "wvt": tile_et(T(inputs["Wv_t"])).astype(BF),
        "wot": tile_et(T(inputs["Wo_t"])).astype(BF),
        "w1t": w1_tiled.astype(BF),
        "w2r": w2_res.astype(BF),
        "gb_c": gb(inputs["g_c"], inputs["b_c"]),
        "gb_t": gb(inputs["g_t"], inputs["b_t"]),
        "gb_m": gb(inputs["g_m"], inputs["b_m"]),
        "b1v": np.asarray(inputs["b1"], np.float32).reshape(F_MLP, 1),
        "b2v": np.asarray(inputs["b2"], np.float32).reshape(D, 1),
    }
    # channel block-diag mask (tokens s-major, groups of 4), tiled 4 qt
    idx = np.arange(128)
    mkc = (idx[:, None] // 4 == idx[None, :] // 4).astype(np.float32)
    shared["mkc4"] = np.tile(mkc, (1, 4)).astype(BF)
    # temporal causal masks: batches of two 128-row k-tiles
    dq = np.arange(TC)
    dk = np.arange(128)
    mkt2 = np.zeros((2, 128, 1024), np.float32)
    for b in range(2):
        for i in range(2):
            r = 2 * b + i
            mkt2[b][:, i * TC:(i + 1) * TC] = (
                dq[None, :] >= r * 128 + dk[:, None]).astype(np.float32)
    shared["mkt2"] = mkt2.astype(BF)

    inv_freq = (10000.0 ** (-np.arange(32, dtype=np.float64) * 2 / HD_T))
    in_maps = []
    for i in range(N_CORES):
        m = dict(shared)
        xs = x[i * SB:(i + 1) * SB].reshape(TL, D)
        m["xT"] = np.ascontiguousarray(xs.T).astype(BF)
        pos = positions[i * SB:(i + 1) * SB].astype(np.float64)
        ang = pos[:, None] * inv_freq[None, :]          # [512, 32]
        cosT = np.cos(ang).T.astype(np.float32)         # [32, 512]
        sinT = np.sin(ang).T.astype(np.float32)
        c4 = np.tile(cosT, (4, 1))                      # [128, 512]
        s4 = np.tile(sinT, (4, 1))
        m["cq"] = np.tile((c4 * 0.125)[:, None, :], (1, 4, 1)).astype(BF)
        m["sq"] = np.tile((s4 * 0.125)[:, None, :], (1, 4, 1)).astype(BF)
        m["ck"] = np.tile(c4[:, None, :], (1, 4, 1)).astype(BF)
        m["sk"] = np.tile(s4[:, None, :], (1, 4, 1)).astype(BF)
        in_maps.append(m)
    return in_maps


def _run(inputs, trace=False):
    from concourse.bass_utils import run_bass_kernel_spmd
    if "prog" not in _CACHE:
        _CACHE["prog"] = _build_program()
    nc, in_names = _CACHE["prog"]
    in_maps = _host_prep(inputs)
    for m in in_maps:
        for k in list(m.keys()):
            assert k in in_names, k
    res = run_bass_kernel_spmd(nc, in_maps, core_ids=list(range(N_CORES)),
                               trace=trace)
    out = np.zeros((S, C, D), np.float32)
    for i in range(N_CORES):
        yT = res.results[i]["yT"]                        # [1024, 2048] c-major
        yi = yT.T.reshape(C, SB, D)                      # [c, s, d]
        out[i * SB:(i + 1) * SB] = yi.transpose(1, 0, 2)
    return out, res


def kernel(**inputs) -> np.ndarray:
    out, _ = _run(inputs, trace=False)
    return out
